# revision 7
# baseline (speedup 1.0000x reference)
"""Trainium2 Bass kernel for nn_GAT_mlp_fed_1gram (3-layer GAT + 1-gram + FFN).

Self-contained: host-side numpy prep (sharding/sorting/index build + small-weight
folding) + an 8-core SPMD Bass/Tile program (graph-parallel slabs, dma_gather of
projected node features from an AllGathered table, one-hot-matmul segment
softmax/scatter, local pooling + FFN), assembled back to the full [128, 2] output.

Algorithm notes (validated against the reference in numpy to ~2.5e-5 rel err):
  - (ee*a_e).sum(-1) folds to edge_attr @ (We . a_e)  -> [72, heads] per layer
  - (xs*a_s).sum(-1) folds into the projection: h @ [W | W.As | W.Ad]
  - segment softmax without max-subtraction (alpha is O(1)), normalization by
    post-division:  out = (sum_e e^a * xs_src) / (sum_e e^a + 1e-16)
  - self-loops (edge_attr fill 'mean') handled as one identity-chunk per tile
"""
import os
import numpy as np
import ml_dtypes

import concourse.bacc as bacc
import concourse.mybir as mybir
import concourse.tile as tile
from concourse.bass_utils import run_bass_kernel_spmd
from concourse.library_config import mlp as _mlp_lib

BF16 = ml_dtypes.bfloat16
F32 = mybir.dt.float32
BF = mybir.dt.bfloat16
I16 = mybir.dt.int16

N, E, G = 50000, 400000, 128
D_NODE, EDGE_DIM, HEADS = 64, 72, 4
H0, H1, H2 = 128, 128, 64
NCLS = 2
NEG = 0.2
NCORES = 8
GPC = G // NCORES
P = 128
BUCKET = 32768
AOFF = {1: 0, 2: 4, 3: 8}        # layer offset into the 9-wide folded edge-alpha
LHEADS = {1: HEADS, 2: HEADS, 3: 1}
LC = {1: HEADS * H0, 2: HEADS * H1, 3: H2}   # output feature width per layer
LDIN = {1: D_NODE, 2: HEADS * H0, 3: HEADS * H1}
# table row slots (bf16): [asrc fp32 (2*heads slots) | xs (C slots)] padded to stride
LROW = {1: 640, 2: 640, 3: 128}
LAS = {1: 8, 2: 8, 3: 2}         # bf16 slots used by fp32 asrc
EXP = mybir.ActivationFunctionType.Exp
RELU = mybir.ActivationFunctionType.Relu
COPY = mybir.ActivationFunctionType.Copy
SQUARE = mybir.ActivationFunctionType.Square
SQRT = mybir.ActivationFunctionType.Sqrt
EQ = mybir.AluOpType.is_equal
MULT = mybir.AluOpType.mult
ADD = mybir.AluOpType.add
MAX = mybir.AluOpType.max


def _wrap16(idx):
    """dma_gather idx layout: idx i -> [i%16, i//16], replicated to 128 partitions."""
    n = len(idx)
    assert n % 16 == 0
    w = np.zeros((16, n // 16), np.int16)
    w[np.arange(n) % 16, np.arange(n) // 16] = idx
    return np.tile(w, (8, 1))


def host_prep(inputs):
    x = np.ascontiguousarray(np.asarray(inputs["x"], np.float32))
    ei = np.asarray(inputs["edge_index"])
    ea = np.ascontiguousarray(np.asarray(inputs["edge_attr"], np.float32))
    batch = np.asarray(inputs["batch"]).astype(np.int64)
    src, dst = ei[0].astype(np.int64), ei[1].astype(np.int64)

    node_start = np.searchsorted(batch, np.arange(0, G + 1, GPC))
    NT = int(np.ceil(np.diff(node_start).max() / P))
    NMAX = NT * P
    core_of_node = np.searchsorted(node_start[1:], np.arange(N), side="right")
    local_of_node = np.arange(N) - node_start[core_of_node]
    table_row = core_of_node * NMAX + local_of_node

    e_core = core_of_node[dst]
    per_core = []
    CA_need = CB_need = 0
    for k in range(NCORES):
        sel = np.nonzero(e_core == k)[0]
        d_loc = local_of_node[dst[sel]]
        order = np.argsort(d_loc, kind="stable")
        sel, d_loc = sel[order], d_loc[order]
        s_row = table_row[src[sel]]
        per_core.append((sel, d_loc, s_row))
        t_of = d_loc // P
        for t in range(NT):
            m = t_of == t
            ca = int((s_row[m] < BUCKET).sum())
            CA_need = max(CA_need, ca)
            CB_need = max(CB_need, int(m.sum()) - ca)
    CPT_A = max(1, int(np.ceil(CA_need / P)))
    CPT_B = max(1, int(np.ceil(CB_need / P)))
    CPT = CPT_A + CPT_B
    CA, CB = CPT_A * P, CPT_B * P

    idx_w = np.zeros((NCORES, NT, 128, (CA + CB) // 16), np.int16)
    mask4 = np.zeros((NCORES, 128, NT * CPT * 4), np.float32)
    dstl = np.zeros((NCORES, 128, NT * CPT), np.float32)
    ea_stream = np.zeros((NCORES, NT * CPT, P, EDGE_DIM + 1), np.float32)
    og_core = (batch[src] // GPC).astype(np.int64)
    NOG = max(int((og_core == k).sum()) for k in range(NCORES))
    NOG = int(np.ceil(NOG / P)) * P
    ea_og = np.zeros((NCORES, NOG // P, P, EDGE_DIM), np.float32)
    gl_og = np.full((NCORES, 128, NOG // P), 200.0, np.float32)

    for k in range(NCORES):
        sel, d_loc, s_row = per_core[k]
        t_of = d_loc // P
        for t in range(NT):
            m = np.nonzero(t_of == t)[0]
            sa = m[s_row[m] < BUCKET]
            sb_ = m[s_row[m] >= BUCKET]
            ia = np.zeros(CA, np.int16)
            ib = np.zeros(CB, np.int16)
            ia[:len(sa)] = s_row[sa].astype(np.int16)
            ib[:len(sb_)] = (s_row[sb_] - BUCKET).astype(np.int16)
            idx_w[k, t] = np.concatenate([_wrap16(ia), _wrap16(ib)], 1)
            for c_off, rows in ((0, sa), (CA, sb_)):
                nn_ = len(rows)
                j = np.arange(nn_)
                cols = (t * CPT * P + c_off + j)
                mask4[k, (cols % P), (cols // P) * 4 + 0] = 1.0
                mask4[k, (cols % P), (cols // P) * 4 + 1] = 1.0
                mask4[k, (cols % P), (cols // P) * 4 + 2] = 1.0
                mask4[k, (cols % P), (cols // P) * 4 + 3] = 1.0
                dstl[k, (cols % P), (cols // P)] = (d_loc[rows] - t * P).astype(np.float32)
                ea_stream[k, cols // P, cols % P, :EDGE_DIM] = ea[sel[rows]]
                ea_stream[k, cols // P, cols % P, EDGE_DIM] = 1.0
        m = np.nonzero(og_core == k)[0]
        j = np.arange(len(m))
        ea_og[k, j // P, j % P] = ea[m]
        gl_og[k, (j % P), (j // P)] = (batch[src[m]] - k * GPC).astype(np.float32)

    def fold(W, a_s, a_d, heads):
        Wr = np.asarray(W, np.float32).reshape(W.shape[0], heads, -1)
        return np.concatenate([np.einsum("dhc,hc->dh", Wr, np.asarray(a_s, np.float32)),
                               np.einsum("dhc,hc->dh", Wr, np.asarray(a_d, np.float32))], 1)

    W_ext = {
        1: np.concatenate([np.asarray(inputs["W1"], np.float32),
                           fold(inputs["W1"], inputs["as1"], inputs["ad1"], HEADS)], 1),
        2: np.concatenate([np.asarray(inputs["W2"], np.float32),
                           fold(inputs["W2"], inputs["as2"], inputs["ad2"], HEADS)], 1),
        3: np.concatenate([np.asarray(inputs["W3"], np.float32),
                           fold(inputs["W3"], inputs["as3"], inputs["ad3"], 1)], 1),
    }
    Wae = np.concatenate([
        np.einsum("dhc,hc->dh", np.asarray(inputs["We1"], np.float32).reshape(EDGE_DIM, HEADS, H0), np.asarray(inputs["ae1"], np.float32)),
        np.einsum("dhc,hc->dh", np.asarray(inputs["We2"], np.float32).reshape(EDGE_DIM, HEADS, H1), np.asarray(inputs["ae2"], np.float32)),
        np.einsum("dhc,hc->dh", np.asarray(inputs["We3"], np.float32).reshape(EDGE_DIM, 1, H2), np.asarray(inputs["ae3"], np.float32)),
    ], 1)  # [72, 9]

    x_T = np.zeros((NCORES, D_NODE, NMAX), np.float32)
    gl_node = np.full((NCORES, 128, NT), 200.0, np.float32)
    inv_cnt = np.zeros((NCORES, 128, GPC), np.float32)
    for k in range(NCORES):
        n0, n1 = node_start[k], node_start[k + 1]
        x_T[k, :, :n1 - n0] = x[n0:n1].T
        loc = np.arange(n1 - n0)
        gl_node[k, loc % P, loc // P] = (batch[n0:n1] - k * GPC).astype(np.float32)
        cnt = np.bincount(batch[n0:n1] - k * GPC, minlength=GPC).astype(np.float32)
        inv_cnt[k] = np.tile(1.0 / np.maximum(cnt, 1.0), (P, 1))

    const = dict(
        iota_row=np.tile(np.arange(P, dtype=np.float32), (P, 1)),
        iota16=np.tile(np.arange(GPC, dtype=np.float32), (P, 1)),
        ident_bf=np.eye(P, dtype=np.float32).astype(BF16),
        ident_f32=np.eye(P, dtype=np.float32),
        Wae=Wae.astype(np.float32),
        W1ext=W_ext[1], W2ext=W_ext[2].reshape(4, 128, 520).transpose(1, 0, 2).reshape(128, 4 * 520).copy(),
        W3ext=W_ext[3].reshape(4, 128, 66).transpose(1, 0, 2).reshape(128, 4 * 66).copy(),
        B1=np.tile(np.asarray(inputs["b1"], np.float32), (P, 1)),
        B2=np.tile(np.asarray(inputs["b2"], np.float32), (P, 1)),
        B3=np.tile(np.asarray(inputs["b3"], np.float32), (P, 1)),
        Wf1a=np.asarray(inputs["Wf1"], np.float32)[:H2],
        Wf1b=np.asarray(inputs["Wf1"], np.float32)[H2:],
        Wf2=np.asarray(inputs["Wf2"], np.float32),
        bf1c=np.asarray(inputs["bf1"], np.float32)[:, None],
        bf2c=np.asarray(inputs["bf2"], np.float32)[:, None],
    )
    dims = dict(NT=NT, NMAX=NMAX, CPT_A=CPT_A, CPT_B=CPT_B, CPT=CPT, CA=CA, CB=CB, NOG=NOG)
    percore = dict(idx_w=idx_w, mask4=mask4, dstl=dstl, ea_stream=ea_stream,
                   ea_og=ea_og, gl_og=gl_og, x_T=x_T, gl_node=gl_node, inv_cnt=inv_cnt)
    return dims, const, percore, node_start


def build_program(dims, const):
    NT, NMAX = dims["NT"], dims["NMAX"]
    CPT_A, CPT_B, CPT = dims["CPT_A"], dims["CPT_B"], dims["CPT"]
    CA, CB = dims["CA"], dims["CB"]
    NOG = dims["NOG"]
    NOGC = NOG // P
    IDXW = (CA + CB) // 16

    nc = bacc.Bacc("TRN2", target_bir_lowering=False, debug=False, num_devices=NCORES)

    # ---------------- DRAM I/O ----------------
    din = {}
    def dram_in(name, shape, dt=F32):
        din[name] = nc.dram_tensor(name, list(shape), dt, kind="ExternalInput")
        return din[name]

    ea_dram = dram_in("ea_stream", [NT * CPT, P, EDGE_DIM + 1])
    ea_og_dram = dram_in("ea_og", [NOGC, P, EDGE_DIM])
    gl_og_dram = dram_in("gl_og", [P, NOGC])
    idx_dram = dram_in("idx_w", [NT, P, IDXW], I16)
    mask4_dram = dram_in("mask4", [P, NT * CPT * 4])
    dstl_dram = dram_in("dstl", [P, NT * CPT])
    xT_dram = dram_in("x_T", [D_NODE, NMAX])
    gl_node_dram = dram_in("gl_node", [P, NT])
    inv_cnt_dram = dram_in("inv_cnt", [P, GPC])
    for cname, arr in const.items():
        dram_in(cname, arr.shape, BF if arr.dtype == BF16 else F32)

    out_dram = nc.dram_tensor("out_gc", [GPC, NCLS], F32, kind="ExternalOutput")

    # internal DRAM
    ag_in = {l: nc.dram_tensor(f"ag_in{l}", [NMAX, LROW[l]], BF, kind="Internal")
             for l in (1, 2, 3)}
    table = {l: nc.dram_tensor(f"table{l}", [NCORES * NMAX, LROW[l]], BF,
                               kind="Internal", addr_space="Shared") for l in (1, 2, 3)}

    RG = [list(range(NCORES))]

    with tile.TileContext(nc) as tc:
        nc.gpsimd.load_library(_mlp_lib)
        import contextlib
        ctx = contextlib.ExitStack()
        with ctx:
            persist = ctx.enter_context(tc.tile_pool(name="persist", bufs=1))
            ppsum = ctx.enter_context(tc.tile_pool(name="ppsum", bufs=1, space="PSUM"))

            # ---------- persistent SBUF ----------
            def pload(name, shape=None, dt=F32):
                t = persist.tile(list(shape if shape is not None else const[name].shape), dt, tag=name)
                nc.sync.dma_start(t[:], din[name][:])
                return t

            iota_row = pload("iota_row")
            iota16 = pload("iota16")
            ident_bf = pload("ident_bf", dt=BF)
            ident_f32 = pload("ident_f32")
            Wae_sb = pload("Wae")
            W1ext_sb = pload("W1ext")
            W2ext_sb = pload("W2ext")
            W3ext_sb = pload("W3ext")
            B_sb = {1: pload("B1"), 2: pload("B2"), 3: pload("B3")}
            Wf1a_sb = pload("Wf1a"); Wf1b_sb = pload("Wf1b"); Wf2_sb = pload("Wf2")
            bf1c_sb = pload("bf1c"); bf2c_sb = pload("bf2c")
            mask4_sb = persist.tile([P, NT * CPT * 4], F32, tag="mask4")
            nc.sync.dma_start(mask4_sb[:], mask4_dram[:])
            dstl_sb = persist.tile([P, NT * CPT], F32, tag="dstl")
            nc.sync.dma_start(dstl_sb[:], dstl_dram[:])
            idx_sb = persist.tile([P, NT * IDXW], I16, tag="idx")
            nc.sync.dma_start(idx_sb[:].rearrange("p (t k) -> p t k", t=NT),
                              idx_dram[:].rearrange("t p k -> p t k"))
            gl_node_sb = persist.tile([P, NT], F32, tag="gl_node")
            nc.sync.dma_start(gl_node_sb[:], gl_node_dram[:])
            inv_cnt_sb = persist.tile([P, GPC], F32, tag="inv_cnt")
            nc.sync.dma_start(inv_cnt_sb[:], inv_cnt_dram[:])
            gl_og_sb = persist.tile([P, NOGC], F32, tag="gl_og")
            nc.sync.dma_start(gl_og_sb[:], gl_og_dram[:])

            alpha_e_sb = persist.tile([P, NT * CPT * 9], F32, tag="alpha_e")
            alpha_loop_sb = persist.tile([P, NT * 9], F32, tag="alpha_loop")
            asd_own = persist.tile([P, NT * 8], F32, tag="asd_own")
            asum_own = persist.tile([P, NT * 4], F32, tag="asum_own")
            ad_bf = persist.tile([P, NT * 4], BF, tag="ad_bf")
            h_slab = persist.tile([P, NT * 512], BF, tag="h_slab")
            og_raw = persist.tile([GPC, EDGE_DIM - 1], F32, tag="og_raw")

            psum_og = ppsum.tile([GPC, EDGE_DIM - 1], F32, tag="og")
            psum_pool = ppsum.tile([H2, GPC], F32, tag="pool")

            # ================= PREPASS =================
            with tc.tile_pool(name="pre", bufs=3) as pre, \
                 tc.tile_pool(name="prepsum", bufs=2, space="PSUM") as pps:
                for t in range(NT):
                    psum_la = pps.tile([P, EDGE_DIM + 1], F32, tag="la")
                    for c in range(CPT):
                        tcn = t * CPT + c
                        eac = pre.tile([P, EDGE_DIM + 1], F32, tag="ea")
                        nc.sync.dma_start(eac[:], ea_dram[tcn])
                        Sf = pre.tile([P, P], F32, tag="sf")
                        nc.vector.tensor_scalar(Sf[:], iota_row[:], dstl_sb[:, tcn:tcn + 1], None, op0=EQ)
                        nc.tensor.matmul(psum_la[:], lhsT=Sf[:], rhs=eac[:],
                                         start=(c == 0), stop=(c == CPT - 1))
                        psum_eaT = pps.tile([EDGE_DIM, P], F32, tag="eaT")
                        nc.tensor.transpose(psum_eaT[:], eac[:, :EDGE_DIM], ident_f32[:])
                        eaT = pre.tile([EDGE_DIM, P], F32, tag="eaTs")
                        nc.scalar.copy(eaT[:], psum_eaT[:])
                        psum_ae = pps.tile([P, 9], F32, tag="ae")
                        nc.tensor.matmul(psum_ae[:], lhsT=eaT[:], rhs=Wae_sb[:], start=True, stop=True)
                        nc.scalar.copy(alpha_e_sb[:, tcn * 9:(tcn + 1) * 9], psum_ae[:])
                    # loop_attr for tile t
                    dmax = pre.tile([P, 1], F32, tag="dmax")
                    nc.vector.tensor_scalar(dmax[:], psum_la[:, EDGE_DIM:EDGE_DIM + 1], 1.0, None, op0=MAX)
                    rd = pre.tile([P, 1], F32, tag="rd")
                    nc.vector.reciprocal(rd[:], dmax[:])
                    la = pre.tile([P, EDGE_DIM], F32, tag="la_sb")
                    nc.vector.tensor_scalar(la[:], psum_la[:, :EDGE_DIM], rd[:], None, op0=MULT)
                    psum_laT = pps.tile([EDGE_DIM, P], F32, tag="eaT")
                    nc.tensor.transpose(psum_laT[:], la[:], ident_f32[:])
                    laT = pre.tile([EDGE_DIM, P], F32, tag="eaTs")
                    nc.scalar.copy(laT[:], psum_laT[:])
                    psum_ael = pps.tile([P, 9], F32, tag="ae")
                    nc.tensor.matmul(psum_ael[:], lhsT=laT[:], rhs=Wae_sb[:], start=True, stop=True)
                    nc.scalar.copy(alpha_loop_sb[:, t * 9:(t + 1) * 9], psum_ael[:])
                # og accumulation
                for oc in range(NOGC):
                    eo = pre.tile([P, EDGE_DIM], F32, tag="eo")
                    nc.sync.dma_start(eo[:], ea_og_dram[oc])
                    Sog = pre.tile([P, GPC], F32, tag="sog")
                    nc.vector.tensor_scalar(Sog[:], iota16[:], gl_og_sb[:, oc:oc + 1], None, op0=EQ)
                    nc.tensor.matmul(psum_og[:], lhsT=Sog[:], rhs=eo[:, :EDGE_DIM - 1],
                                     start=(oc == 0), stop=(oc == NOGC - 1))
                nc.scalar.copy(og_raw[:], psum_og[:])

            # ================= LAYERS =================
            for l in (1, 2, 3):
                heads, C, ROW, ASL = LHEADS[l], LC[l], LROW[l], LAS[l]
                NH = C // heads if l < 3 else C   # per-head width (128/128/64)
                HW = C // heads                  # head block width
                # ---------- projection -> ag_in[l] ----------
                with tc.tile_pool(name=f"proj{l}", bufs=2) as pj, \
                     tc.tile_pool(name=f"projp{l}", bufs=2, space="PSUM") as pjp:
                    for t in range(NT):
                        psum_x = pjp.tile([P, C], F32, tag="px")
                        psum_a = pjp.tile([P, 2 * heads], F32, tag="pa")
                        if l == 1:
                            xt = pj.tile([D_NODE, P], F32, tag="xt")
                            nc.sync.dma_start(xt[:], xT_dram[:, t * P:(t + 1) * P])
                            nc.tensor.matmul(psum_x[:], lhsT=xt[:], rhs=W1ext_sb[:, :C], start=True, stop=True)
                            nc.tensor.matmul(psum_a[:], lhsT=xt[:], rhs=W1ext_sb[:, C:C + 2 * heads], start=True, stop=True)
                        else:
                            Wsb = W2ext_sb if l == 2 else W3ext_sb
                            WR = 520 if l == 2 else 66
                            for kb in range(4):
                                psum_hT = pjp.tile([P, P], BF, tag="phT")
                                nc.tensor.transpose(psum_hT[:], h_slab[:, t * 512 + kb * 128: t * 512 + (kb + 1) * 128], ident_bf[:])
                                hT = pj.tile([P, P], F32, tag="hT")
                                nc.scalar.copy(hT[:], psum_hT[:])
                                nc.tensor.matmul(psum_x[:], lhsT=hT[:], rhs=Wsb[:, kb * WR:kb * WR + C],
                                                 start=(kb == 0), stop=(kb == 3))
                                nc.tensor.matmul(psum_a[:], lhsT=hT[:], rhs=Wsb[:, kb * WR + C:kb * WR + C + 2 * heads],
                                                 start=(kb == 0), stop=(kb == 3))
                        # own-slab alpha bookkeeping (asrc at +0, adst at +4 regardless of heads)
                        nc.scalar.copy(asd_own[:, t * 8:t * 8 + heads], psum_a[:, :heads])
                        nc.scalar.copy(asd_own[:, t * 8 + 4:t * 8 + 4 + heads], psum_a[:, heads:2 * heads])
                        nc.vector.tensor_copy(ad_bf[:, t * 4:t * 4 + heads], psum_a[:, heads:2 * heads])
                        # table row: [asrc fp32 bitcast | xs bf16]
                        row = pj.tile([P, ROW], BF, tag="row")
                        row_f32 = row[:].bitcast(F32)
                        nc.vector.tensor_copy(row_f32[:, :heads], psum_a[:, :heads])
                        nc.scalar.copy(row[:, ASL:ASL + C], psum_x[:])
                        nc.sync.dma_start(ag_in[l][t * P:(t + 1) * P, :], row[:])
                    # asum = asrc + adst (for self-loop chunks)
                    nc.vector.tensor_tensor(
                        out=asum_own[:].rearrange("p (t k) -> p t k", k=4)[:, :, :heads],
                        in0=asd_own[:].rearrange("p (t k) -> p t k", k=8)[:, :, :heads],
                        in1=asd_own[:].rearrange("p (t k) -> p t k", k=8)[:, :, 4:4 + heads],
                        op=ADD)

                # ---------- AllGather ----------
                nc.gpsimd.collective_compute(
                    "AllGather", mybir.AluOpType.bypass, replica_groups=RG,
                    ins=[ag_in[l][:]], outs=[table[l][:]],
                )

                # ---------- main pass ----------
                NCHUNK = CPT + 1
                AW = heads * NCHUNK           # alpha columns per tile
                with tc.tile_pool(name=f"main{l}", bufs=2) as mn, \
                     tc.tile_pool(name=f"sbuf_s{l}", bufs=NCHUNK + 2) as spool, \
                     tc.tile_pool(name=f"mainp{l}", bufs=2, space="PSUM") as mp:
                    for t in range(NT):
                        gbuf = mn.tile([P, NCHUNK * ROW], BF, tag="gbuf")
                        nc.gpsimd.dma_gather(
                            gbuf[:, :CPT_A * ROW].rearrange("p (c e) -> p c e", e=ROW),
                            table[l][:], idx_sb[:, t * IDXW: t * IDXW + CA // 16],
                            CA, CA, ROW)
                        nc.gpsimd.dma_gather(
                            gbuf[:, CPT_A * ROW:CPT * ROW].rearrange("p (c e) -> p c e", e=ROW),
                            table[l][BUCKET:, :], idx_sb[:, t * IDXW + CA // 16: t * IDXW + IDXW],
                            CB, CB, ROW)
                        nc.sync.dma_start(gbuf[:, CPT * ROW:], ag_in[l][t * P:(t + 1) * P, :])

                        psum_za = mp.tile([P, AW + heads], F32, tag="za")  # [z (heads) | ad (AW)]
                        S_tiles = []
                        for c in range(CPT):
                            tcn = t * CPT + c
                            Sb = spool.tile([P, P], BF, tag="S")
                            nc.vector.tensor_scalar(Sb[:], iota_row[:], dstl_sb[:, tcn:tcn + 1], None, op0=EQ)
                            S_tiles.append(Sb)
                            psum_ST = mp.tile([P, P], BF, tag="st")
                            nc.tensor.transpose(psum_ST[:], Sb[:], ident_bf[:])
                            STs = mn.tile([P, P], BF, tag="sts")
                            nc.scalar.copy(STs[:], psum_ST[:])
                            nc.tensor.matmul(psum_za[:, heads + c * heads: heads + (c + 1) * heads],
                                             lhsT=STs[:], rhs=ad_bf[:, t * 4:t * 4 + heads],
                                             start=True, stop=True)
                        # ---- alpha assembly for the whole tile ----
                        t_al = mn.tile([P, AW], F32, tag="t_al")
                        gb_f32 = gbuf[:].bitcast(F32).rearrange("p (c e) -> p c e", e=ROW // 2)
                        nc.vector.tensor_tensor(
                            out=t_al[:].rearrange("p (c k) -> p c k", k=heads)[:, :CPT, :],
                            in0=gb_f32[:, :CPT, :heads],
                            in1=alpha_e_sb[:, t * CPT * 9:(t + 1) * CPT * 9].rearrange(
                                "p (c k) -> p c k", k=9)[:, :, AOFF[l]:AOFF[l] + heads],
                            op=ADD)
                        nc.vector.tensor_tensor(
                            out=t_al[:, CPT * heads:],
                            in0=asum_own[:, t * 4:t * 4 + heads],
                            in1=alpha_loop_sb[:, t * 9 + AOFF[l]: t * 9 + AOFF[l] + heads],
                            op=ADD)
                        nc.vector.tensor_tensor(out=t_al[:, :CPT * heads], in0=t_al[:, :CPT * heads],
                                                in1=psum_za[:, heads:heads + CPT * heads], op=ADD)
                        t_lr = mn.tile([P, AW], F32, tag="t_lr")
                        nc.vector.tensor_scalar(t_lr[:], t_al[:], NEG, None, op0=MULT)
                        nc.vector.tensor_tensor(out=t_al[:], in0=t_al[:], in1=t_lr[:], op=MAX)
                        p_f = mn.tile([P, AW], F32, tag="p_f")
                        nc.scalar.activation(p_f[:], t_al[:], EXP)
                        if heads == 4:
                            mview = mask4_sb[:, t * CPT * 4:(t + 1) * CPT * 4]
                        else:
                            mview = mask4_sb[:].rearrange("p (n k) -> p n k", k=4)[:, t * CPT:(t + 1) * CPT, 0]
                        nc.vector.tensor_tensor(out=p_f[:, :CPT * heads], in0=p_f[:, :CPT * heads],
                                                in1=mview, op=MULT)
                        p_bf = mn.tile([P, AW], BF, tag="p_bf")
                        nc.vector.tensor_copy(p_bf[:], p_f[:])
                        # ---- message matmuls ----
                        psum_M = mp.tile([P, C], F32, tag="M")
                        for c in range(NCHUNK):
                            Sb = S_tiles[c] if c < CPT else ident_bf
                            g_xs = gbuf[:, c * ROW + ASL: c * ROW + ASL + C]
                            m_t = mn.tile([P, C], BF, tag="m")
                            for h in range(heads):
                                sc = p_f[:, c * heads + h: c * heads + h + 1]
                                dst_sl = m_t[:, h * HW:(h + 1) * HW]
                                src_sl = g_xs[:, h * HW:(h + 1) * HW] if l < 3 else g_xs
                                if h < heads // 2 or heads == 1:
                                    nc.scalar.activation(dst_sl, src_sl, COPY, scale=sc)
                                else:
                                    nc.vector.tensor_scalar(dst_sl, src_sl, sc, None, op0=MULT)
                            nc.tensor.matmul(psum_M[:], lhsT=Sb[:], rhs=m_t[:],
                                             start=(c == 0), stop=(c == NCHUNK - 1))
                            nc.tensor.matmul(psum_za[:, :heads], lhsT=Sb[:],
                                             rhs=p_bf[:, c * heads:(c + 1) * heads],
                                             start=(c == 0), stop=(c == NCHUNK - 1))
                        # ---- epilogue ----
                        zt = mn.tile([P, heads], F32, tag="zt")
                        nc.vector.tensor_scalar(zt[:], psum_za[:, :heads], 1e-16, None, op0=ADD)
                        rz = mn.tile([P, heads], F32, tag="rz")
                        nc.vector.reciprocal(rz[:], zt[:])
                        ht = mn.tile([P, C], F32, tag="ht")
                        for h in range(heads):
                            nc.vector.tensor_scalar(ht[:, h * HW:(h + 1) * HW],
                                                    psum_M[:, h * HW:(h + 1) * HW],
                                                    rz[:, h:h + 1], None, op0=MULT)
                        nc.vector.tensor_tensor(out=ht[:], in0=ht[:], in1=B_sb[l][:, :C], op=ADD)
                        if l < 3:
                            nc.scalar.activation(h_slab[:, t * 512:(t + 1) * 512], ht[:], RELU)
                        else:
                            h3 = mn.tile([P, C], F32, tag="h3")
                            nc.scalar.activation(h3[:], ht[:], RELU)
                            Sp = mn.tile([P, GPC], F32, tag="Sp")
                            nc.vector.tensor_scalar(Sp[:], iota16[:], gl_node_sb[:, t:t + 1], None, op0=EQ)
                            nc.vector.tensor_tensor(out=Sp[:], in0=Sp[:], in1=inv_cnt_sb[:], op=MULT)
                            nc.tensor.matmul(psum_pool[:], lhsT=h3[:], rhs=Sp[:],
                                             start=(t == 0), stop=(t == NT - 1))

            # ================= FINAL: og norm + FFN + softmax =================
            with tc.tile_pool(name="fin", bufs=1) as fin, \
                 tc.tile_pool(name="finp", bufs=1, space="PSUM") as fnp:
                sq = fin.tile([GPC, EDGE_DIM - 1], F32, tag="sq")
                nc.scalar.activation(sq[:], og_raw[:], SQUARE)
                ss = fin.tile([GPC, 1], F32, tag="ss")
                nc.vector.tensor_reduce(out=ss[:], in_=sq[:], op=ADD, axis=mybir.AxisListType.X)
                nc.vector.tensor_scalar(ss[:], ss[:], 1e-24, None, op0=MAX)
                iss = fin.tile([GPC, 1], F32, tag="iss")
                nc.vector.reciprocal(iss[:], ss[:])
                rs = fin.tile([GPC, 1], F32, tag="rs")
                nc.scalar.activation(rs[:], iss[:], SQRT)
                ogn = fin.tile([GPC, EDGE_DIM - 1], F32, tag="ogn")
                nc.vector.tensor_scalar(ogn[:], og_raw[:], rs[:], None, op0=MULT)
                psum_ogT = fnp.tile([EDGE_DIM - 1, GPC], F32, tag="ogT")
                nc.tensor.transpose(psum_ogT[:], ogn[:], ident_f32[:GPC, :GPC])
                ogT = fin.tile([EDGE_DIM - 1, GPC], F32, tag="ogTs")
                nc.scalar.copy(ogT[:], psum_ogT[:])
                pooledT = fin.tile([H2, GPC], F32, tag="pooledT")
                nc.scalar.copy(pooledT[:], psum_pool[:])
                psum_z1 = fnp.tile([67, GPC], F32, tag="z1")
                nc.tensor.matmul(psum_z1[:], lhsT=Wf1a_sb[:], rhs=pooledT[:], start=True, stop=False)
                nc.tensor.matmul(psum_z1[:], lhsT=Wf1b_sb[:], rhs=ogT[:], start=False, stop=True)
                z1 = fin.tile([67, GPC], F32, tag="z1s")
                nc.scalar.activation(z1[:], psum_z1[:], RELU, bias=bf1c_sb[:])
                psum_z2 = fnp.tile([NCLS, GPC], F32, tag="z2")
                nc.tensor.matmul(psum_z2[:], lhsT=Wf2_sb[:], rhs=z1[:], start=True, stop=True)
                z2b = fin.tile([NCLS, GPC], F32, tag="z2b")
                nc.scalar.activation(z2b[:], psum_z2[:], mybir.ActivationFunctionType.Identity, bias=bf2c_sb[:])
                psum_z2T = fnp.tile([GPC, NCLS], F32, tag="z2T")
                nc.tensor.transpose(psum_z2T[:], z2b[:], ident_f32[:NCLS, :NCLS])
                e2 = fin.tile([GPC, NCLS], F32, tag="e2")
                nc.scalar.activation(e2[:], psum_z2T[:], EXP)
                s2 = fin.tile([GPC, 1], F32, tag="s2")
                nc.vector.tensor_reduce(out=s2[:], in_=e2[:], axis=mybir.AxisListType.X, op=ADD)
                r2 = fin.tile([GPC, 1], F32, tag="r2")
                nc.vector.reciprocal(r2[:], s2[:])
                o2 = fin.tile([GPC, NCLS], F32, tag="o2")
                nc.vector.tensor_scalar(o2[:], e2[:], r2[:], None, op0=MULT)
                nc.sync.dma_start(out_dram[:], o2[:])

    nc.compile()
    return nc


def kernel(**inputs) -> np.ndarray:
    dims, const, percore, node_start = host_prep(inputs)
    nc = build_program(dims, const)
    in_maps = []
    for k in range(NCORES):
        m = {name: np.ascontiguousarray(arr) for name, arr in const.items()}
        m.update(
            ea_stream=percore["ea_stream"][k],
            ea_og=percore["ea_og"][k],
            gl_og=percore["gl_og"][k],
            idx_w=percore["idx_w"][k],
            mask4=percore["mask4"][k],
            dstl=percore["dstl"][k],
            x_T=percore["x_T"][k],
            gl_node=percore["gl_node"][k],
            inv_cnt=percore["inv_cnt"][k],
        )
        in_maps.append(m)
    trace = bool(int(os.environ.get("BASS_KERNEL_TRACE", "0")))
    if trace:
        # the image's antenv lacks axon_hooks; inject it from trn_agent_boot
        try:
            import sys as _sys, types as _types
            if "antenv.axon_hooks" not in _sys.modules:
                _m = _types.ModuleType("antenv.axon_hooks")
                _h = [None]

                def _get():
                    if _h[0] is None:
                        from trn_agent_boot.trn_boot import _ntff_profile_via_ctypes
                        _h[0] = _ntff_profile_via_ctypes("/opt/axon/libaxon_pjrt.so")
                    return _h[0]

                _m.get_axon_ntff_profile_hook = _get
                _m.set_axon_ntff_profile_hook = lambda h: _h.__setitem__(0, h)
                _sys.modules["antenv.axon_hooks"] = _m
        except Exception:
            trace = False
    res = run_bass_kernel_spmd(nc, in_maps, core_ids=list(range(NCORES)), trace=trace)
    if trace and res.exec_time_ns is not None:
        print(f"HW exec time: {res.exec_time_ns} ns")
    out = np.zeros((G, NCLS), np.float32)
    for k in range(NCORES):
        out[k * GPC:(k + 1) * GPC] = np.asarray(res.results[k]["out_gc"], np.float32)
    return out


# revision 10
# speedup vs baseline: 1.2200x; 1.2200x over previous
"""Trainium2 Bass kernel for nn_GAT_mlp_fed_1gram (3-layer GAT + 1-gram + FFN).

Self-contained: host-side numpy prep (sharding/sorting/index build + small-weight
folding) + an 8-core SPMD Bass/Tile program (graph-parallel slabs, dma_gather of
projected node features from an AllGathered table, one-hot-matmul segment
softmax/scatter, local pooling + FFN), assembled back to the full [128, 2] output.

Algorithm notes (validated against the reference in numpy):
  - (ee*a_e).sum(-1) folds to edge_attr @ (We . a_e)  -> [72, heads] per layer
  - (xs*a_s).sum(-1) folds into the projection: h @ [W | W.As | W.Ad]
  - segment softmax without max-subtraction (alpha is O(1)), normalization by
    post-division:  out = (sum_e e^a * xs_src) / (sum_e e^a + 1e-16)
  - loop_attr @ Wae == segment_mean(edge alphas): self-loop alphas come from a
    segment-mean of the per-edge folded alphas (matmul commutes with seg-sum)
  - self-loops (edge_attr fill 'mean') handled as one identity-chunk per tile
  - one-hot scatter matrices S / S^T are built once in the prepass and cached
    in DRAM; all three layers stream them back
"""
import os
import numpy as np
import ml_dtypes

import concourse.bacc as bacc
import concourse.mybir as mybir
import concourse.tile as tile
from concourse.bass_utils import run_bass_kernel_spmd
from concourse.library_config import mlp as _mlp_lib

BF16 = ml_dtypes.bfloat16
F32 = mybir.dt.float32
BF = mybir.dt.bfloat16
I16 = mybir.dt.int16

N, E, G = 50000, 400000, 128
D_NODE, EDGE_DIM, HEADS = 64, 72, 4
H0, H1, H2 = 128, 128, 64
NCLS = 2
NEG = 0.2
NCORES = 8
GPC = G // NCORES
P = 128
BUCKET = 32768
AOFF = {1: 0, 2: 4, 3: 8}
LHEADS = {1: HEADS, 2: HEADS, 3: 1}
LC = {1: HEADS * H0, 2: HEADS * H1, 3: H2}
LROW = {1: 640, 2: 640, 3: 128}   # bf16 slots per table row (stride, 256B mult)
LAS = {1: 8, 2: 8, 3: 2}          # leading bf16 slots holding fp32 asrc
EXP = mybir.ActivationFunctionType.Exp
RELU = mybir.ActivationFunctionType.Relu
COPY = mybir.ActivationFunctionType.Copy
SQUARE = mybir.ActivationFunctionType.Square
SQRT = mybir.ActivationFunctionType.Sqrt
IDENT = mybir.ActivationFunctionType.Identity
EQ = mybir.AluOpType.is_equal
MULT = mybir.AluOpType.mult
ADD = mybir.AluOpType.add
MAX = mybir.AluOpType.max


def _wrap16(idx):
    """dma_gather idx layout: idx i -> [i%16, i//16], replicated to 128 partitions."""
    n = len(idx)
    assert n % 16 == 0
    w = np.zeros((16, n // 16), np.int16)
    w[np.arange(n) % 16, np.arange(n) // 16] = idx
    return np.tile(w, (8, 1))


def host_prep(inputs):
    x = np.ascontiguousarray(np.asarray(inputs["x"], np.float32))
    ei = np.asarray(inputs["edge_index"])
    ea = np.ascontiguousarray(np.asarray(inputs["edge_attr"], np.float32))
    batch = np.asarray(inputs["batch"]).astype(np.int64)
    src, dst = ei[0].astype(np.int64), ei[1].astype(np.int64)

    node_start = np.searchsorted(batch, np.arange(0, G + 1, GPC))
    NT = int(np.ceil(np.diff(node_start).max() / P))
    NMAX = NT * P
    core_of_node = np.searchsorted(node_start[1:], np.arange(N), side="right")
    local_of_node = np.arange(N) - node_start[core_of_node]
    table_row = core_of_node * NMAX + local_of_node

    e_core = core_of_node[dst]
    per_core = []
    CA_need = CB_need = 0
    for k in range(NCORES):
        sel = np.nonzero(e_core == k)[0]
        d_loc = local_of_node[dst[sel]]
        order = np.argsort(d_loc, kind="stable")
        sel, d_loc = sel[order], d_loc[order]
        s_row = table_row[src[sel]]
        per_core.append((sel, d_loc, s_row))
        t_of = d_loc // P
        for t in range(NT):
            m = t_of == t
            ca = int((s_row[m] < BUCKET).sum())
            CA_need = max(CA_need, ca)
            CB_need = max(CB_need, int(m.sum()) - ca)
    CPT_A = max(1, int(np.ceil(CA_need / P)))
    CPT_B = max(1, int(np.ceil(CB_need / P)))
    CPT = CPT_A + CPT_B
    CA, CB = CPT_A * P, CPT_B * P

    idx_w = np.zeros((NCORES, NT, 128, (CA + CB) // 16), np.int16)
    mask4 = np.zeros((NCORES, 128, NT * CPT * 4), np.float32)
    dstl = np.zeros((NCORES, 128, NT * CPT), np.float32)
    eaT_stream = np.zeros((NCORES, NT * CPT, EDGE_DIM, P), BF16)
    og_core = (batch[src] // GPC).astype(np.int64)
    NOG = max(int((og_core == k).sum()) for k in range(NCORES))
    NOG = int(np.ceil(NOG / P)) * P
    ea_og = np.zeros((NCORES, NOG // P, P, EDGE_DIM), BF16)
    gl_og = np.full((NCORES, 128, NOG // P), 200.0, np.float32)

    ea_bf = ea.astype(BF16)
    for k in range(NCORES):
        sel, d_loc, s_row = per_core[k]
        t_of = d_loc // P
        for t in range(NT):
            m = np.nonzero(t_of == t)[0]
            sa = m[s_row[m] < BUCKET]
            sb_ = m[s_row[m] >= BUCKET]
            ia = np.zeros(CA, np.int16)
            ib = np.zeros(CB, np.int16)
            ia[:len(sa)] = s_row[sa].astype(np.int16)
            ib[:len(sb_)] = (s_row[sb_] - BUCKET).astype(np.int16)
            idx_w[k, t] = np.concatenate([_wrap16(ia), _wrap16(ib)], 1)
            for c_off, rows in ((0, sa), (CA, sb_)):
                nn_ = len(rows)
                j = np.arange(nn_)
                cols = (t * CPT * P + c_off + j)
                for hh in range(4):
                    mask4[k, (cols % P), (cols // P) * 4 + hh] = 1.0
                dstl[k, (cols % P), (cols // P)] = (d_loc[rows] - t * P).astype(np.float32)
                eaT_stream[k, cols // P, :, cols % P] = ea_bf[sel[rows]]
        m = np.nonzero(og_core == k)[0]
        j = np.arange(len(m))
        ea_og[k, j // P, j % P] = ea_bf[m]
        gl_og[k, (j % P), (j // P)] = (batch[src[m]] - k * GPC).astype(np.float32)

    def fold(W, a_s, a_d, heads):
        Wr = np.asarray(W, np.float32).reshape(W.shape[0], heads, -1)
        return np.concatenate([np.einsum("dhc,hc->dh", Wr, np.asarray(a_s, np.float32)),
                               np.einsum("dhc,hc->dh", Wr, np.asarray(a_d, np.float32))], 1)

    W_ext = {
        1: np.concatenate([np.asarray(inputs["W1"], np.float32),
                           fold(inputs["W1"], inputs["as1"], inputs["ad1"], HEADS)], 1),
        2: np.concatenate([np.asarray(inputs["W2"], np.float32),
                           fold(inputs["W2"], inputs["as2"], inputs["ad2"], HEADS)], 1),
        3: np.concatenate([np.asarray(inputs["W3"], np.float32),
                           fold(inputs["W3"], inputs["as3"], inputs["ad3"], 1)], 1),
    }
    Wae = np.concatenate([
        np.einsum("dhc,hc->dh", np.asarray(inputs["We1"], np.float32).reshape(EDGE_DIM, HEADS, H0), np.asarray(inputs["ae1"], np.float32)),
        np.einsum("dhc,hc->dh", np.asarray(inputs["We2"], np.float32).reshape(EDGE_DIM, HEADS, H1), np.asarray(inputs["ae2"], np.float32)),
        np.einsum("dhc,hc->dh", np.asarray(inputs["We3"], np.float32).reshape(EDGE_DIM, 1, H2), np.asarray(inputs["ae3"], np.float32)),
    ], 1)  # [72, 9]

    x_T = np.zeros((NCORES, D_NODE, NMAX), np.float32)
    gl_node = np.full((NCORES, 128, NT), 200.0, np.float32)
    inv_cnt = np.zeros((NCORES, 128, GPC), np.float32)
    for k in range(NCORES):
        n0, n1 = node_start[k], node_start[k + 1]
        x_T[k, :, :n1 - n0] = x[n0:n1].T
        loc = np.arange(n1 - n0)
        gl_node[k, loc % P, loc // P] = (batch[n0:n1] - k * GPC).astype(np.float32)
        cnt = np.bincount(batch[n0:n1] - k * GPC, minlength=GPC).astype(np.float32)
        inv_cnt[k] = np.tile(1.0 / np.maximum(cnt, 1.0), (P, 1))

    const = dict(
        iota_row=np.tile(np.arange(P, dtype=np.float32), (P, 1)),
        iota16=np.tile(np.arange(GPC, dtype=np.float32), (P, 1)),
        ident_bf=np.eye(P, dtype=np.float32).astype(BF16),
        ident_f32=np.eye(P, dtype=np.float32),
        Wae=Wae.astype(BF16),
        W1ext=W_ext[1].astype(np.float32),
        W2ext=W_ext[2].reshape(4, 128, 520).transpose(1, 0, 2).reshape(128, 4 * 520).astype(BF16),
        W3ext=W_ext[3].reshape(4, 128, 66).transpose(1, 0, 2).reshape(128, 4 * 66).astype(BF16),
        B1=np.tile(np.asarray(inputs["b1"], np.float32), (P, 1)),
        B2=np.tile(np.asarray(inputs["b2"], np.float32), (P, 1)),
        B3=np.tile(np.asarray(inputs["b3"], np.float32), (P, 1)),
        Wf1a=np.asarray(inputs["Wf1"], np.float32)[:H2],
        Wf1b=np.asarray(inputs["Wf1"], np.float32)[H2:],
        Wf2=np.asarray(inputs["Wf2"], np.float32),
        bf1c=np.asarray(inputs["bf1"], np.float32)[:, None],
        bf2c=np.asarray(inputs["bf2"], np.float32)[:, None],
    )
    dims = dict(NT=NT, NMAX=NMAX, CPT_A=CPT_A, CPT_B=CPT_B, CPT=CPT, CA=CA, CB=CB, NOG=NOG)
    percore = dict(idx_w=idx_w, mask4=mask4, dstl=dstl, eaT_stream=eaT_stream,
                   ea_og=ea_og, gl_og=gl_og, x_T=x_T, gl_node=gl_node, inv_cnt=inv_cnt)
    return dims, const, percore, node_start


def build_program(dims, const):
    NT, NMAX = dims["NT"], dims["NMAX"]
    CPT_A, CPT_B, CPT = dims["CPT_A"], dims["CPT_B"], dims["CPT"]
    CA, CB = dims["CA"], dims["CB"]
    NOG = dims["NOG"]
    NOGC = NOG // P
    IDXW = (CA + CB) // 16
    SB = CPT * P                       # S-block width per tile

    nc = bacc.Bacc("TRN2", target_bir_lowering=False, debug=False, num_devices=NCORES)

    din = {}
    def dram_in(name, shape, dt=F32):
        din[name] = nc.dram_tensor(name, list(shape), dt, kind="ExternalInput")
        return din[name]

    eaT_dram = dram_in("eaT_stream", [NT * CPT, EDGE_DIM, P], BF)
    ea_og_dram = dram_in("ea_og", [NOGC, P, EDGE_DIM], BF)
    gl_og_dram = dram_in("gl_og", [P, NOGC])
    idx_dram = dram_in("idx_w", [NT, P, IDXW], I16)
    mask4_dram = dram_in("mask4", [P, NT * CPT * 4])
    dstl_dram = dram_in("dstl", [P, NT * CPT])
    xT_dram = dram_in("x_T", [D_NODE, NMAX])
    gl_node_dram = dram_in("gl_node", [P, NT])
    inv_cnt_dram = dram_in("inv_cnt", [P, GPC])
    for cname, arr in const.items():
        dram_in(cname, arr.shape, BF if arr.dtype == BF16 else F32)

    out_dram = nc.dram_tensor("out_gc", [GPC, NCLS], F32, kind="ExternalOutput")

    ag_in = {l: nc.dram_tensor(f"ag_in{l}", [NMAX, LROW[l]], BF, kind="Internal")
             for l in (1, 2, 3)}
    table = {l: nc.dram_tensor(f"table{l}", [NCORES * NMAX, LROW[l]], BF,
                               kind="Internal", addr_space="Shared") for l in (1, 2, 3)}
    s_dram = nc.dram_tensor("s_blocks", [NT, P, SB], BF, kind="Internal")
    st_dram = nc.dram_tensor("st_blocks", [NT, P, SB], BF, kind="Internal")

    RG = [list(range(NCORES))]

    with tile.TileContext(nc) as tc:
        nc.gpsimd.load_library(_mlp_lib)
        import contextlib
        ctx = contextlib.ExitStack()
        with ctx:
            persist = ctx.enter_context(tc.tile_pool(name="persist", bufs=1))

            def pload(name, shape=None, dt=F32):
                t = persist.tile(list(shape if shape is not None else const[name].shape), dt, tag=name)
                nc.sync.dma_start(t[:], din[name][:])
                return t

            iota_row = pload("iota_row")
            iota16 = pload("iota16")
            ident_bf = pload("ident_bf", dt=BF)
            ident_f32 = pload("ident_f32")
            Wae_sb = pload("Wae", dt=BF)
            W1ext_sb = pload("W1ext")
            W2ext_sb = pload("W2ext", dt=BF)
            W3ext_sb = pload("W3ext", dt=BF)
            B_sb = {1: pload("B1"), 2: pload("B2"), 3: pload("B3")}
            Wf1a_sb = pload("Wf1a"); Wf1b_sb = pload("Wf1b"); Wf2_sb = pload("Wf2")
            bf1c_sb = pload("bf1c"); bf2c_sb = pload("bf2c")
            mask4_sb = persist.tile([P, NT * CPT * 4], F32, tag="mask4")
            nc.sync.dma_start(mask4_sb[:], mask4_dram[:])
            dstl_sb = persist.tile([P, NT * CPT], F32, tag="dstl")
            nc.sync.dma_start(dstl_sb[:], dstl_dram[:])
            idx_sb = persist.tile([P, NT * IDXW], I16, tag="idx")
            nc.sync.dma_start(idx_sb[:].rearrange("p (t k) -> p t k", t=NT),
                              idx_dram[:].rearrange("t p k -> p t k"))
            gl_node_sb = persist.tile([P, NT], F32, tag="gl_node")
            nc.sync.dma_start(gl_node_sb[:], gl_node_dram[:])
            inv_cnt_sb = persist.tile([P, GPC], F32, tag="inv_cnt")
            nc.sync.dma_start(inv_cnt_sb[:], inv_cnt_dram[:])
            gl_og_sb = persist.tile([P, NOGC], F32, tag="gl_og")
            nc.sync.dma_start(gl_og_sb[:], gl_og_dram[:])

            alpha_e_sb = persist.tile([P, NT * CPT * 9], F32, tag="alpha_e")
            alpha_loop_sb = persist.tile([P, NT * 9], F32, tag="alpha_loop")
            asd_own = persist.tile([P, NT * 8], F32, tag="asd_own")
            asum_own = persist.tile([P, NT * 4], F32, tag="asum_own")
            ad_bf = persist.tile([P, NT * 4], BF, tag="ad_bf")
            h_slab = persist.tile([P, NT * 512], BF, tag="h_slab")
            og_raw = persist.tile([GPC, EDGE_DIM - 1], F32, tag="og_raw")

            # ================= PREPASS =================
            with tc.tile_pool(name="pre", bufs=3) as pre, \
                 tc.tile_pool(name="sblk", bufs=2) as sblk, \
                 tc.tile_pool(name="prepsum", bufs=2, space="PSUM") as pps, \
                 tc.tile_pool(name="ogpsum", bufs=1, space="PSUM") as ogp:
                psum_og = ogp.tile([GPC, EDGE_DIM - 1], F32, tag="og")
                for oc in range(NOGC):
                    eo = pre.tile([P, EDGE_DIM], BF, tag="eo")
                    nc.sync.dma_start(eo[:], ea_og_dram[oc])
                    Sog = pre.tile([P, GPC], BF, tag="sog")
                    nc.vector.tensor_scalar(Sog[:], iota16[:], gl_og_sb[:, oc:oc + 1], None, op0=EQ)
                    nc.tensor.matmul(psum_og[:], lhsT=Sog[:], rhs=eo[:, :EDGE_DIM - 1],
                                     start=(oc == 0), stop=(oc == NOGC - 1))
                nc.scalar.copy(og_raw[:], psum_og[:])

                for t in range(NT):
                    # --- build S block + S^T block, cache to DRAM ---
                    s_blk = sblk.tile([P, SB], BF, tag="s")
                    st_blk = sblk.tile([P, SB], BF, tag="st")
                    for c in range(CPT):
                        tcn = t * CPT + c
                        nc.vector.tensor_scalar(s_blk[:, c * P:(c + 1) * P], iota_row[:],
                                                dstl_sb[:, tcn:tcn + 1], None, op0=EQ)
                        psum_ST = pps.tile([P, P], BF, tag="stp")
                        nc.tensor.transpose(psum_ST[:], s_blk[:, c * P:(c + 1) * P], ident_bf[:])
                        nc.scalar.copy(st_blk[:, c * P:(c + 1) * P], psum_ST[:])
                    nc.sync.dma_start(s_dram[t], s_blk[:])
                    nc.sync.dma_start(st_dram[t], st_blk[:])
                    # --- per-chunk folded edge alphas + segment-mean for loops ---
                    psum_agg = pps.tile([P, 10], F32, tag="agg")
                    for c in range(CPT):
                        tcn = t * CPT + c
                        eaT = pre.tile([EDGE_DIM, P], BF, tag="eaT")
                        nc.sync.dma_start(eaT[:], eaT_dram[tcn])
                        psum_ae = pps.tile([P, 9], F32, tag="ae")
                        nc.tensor.matmul(psum_ae[:], lhsT=eaT[:], rhs=Wae_sb[:], start=True, stop=True)
                        nc.scalar.copy(alpha_e_sb[:, tcn * 9:(tcn + 1) * 9], psum_ae[:])
                        aggrhs = pre.tile([P, 10], BF, tag="aggrhs")
                        nc.scalar.copy(aggrhs[:, :9], psum_ae[:])
                        nc.vector.tensor_copy(aggrhs[:, 9:10],
                                              mask4_sb[:, tcn * 4:tcn * 4 + 1])
                        nc.tensor.matmul(psum_agg[:], lhsT=s_blk[:, c * P:(c + 1) * P],
                                         rhs=aggrhs[:], start=(c == 0), stop=(c == CPT - 1))
                    dmax = pre.tile([P, 1], F32, tag="dmax")
                    nc.vector.tensor_scalar(dmax[:], psum_agg[:, 9:10], 1.0, None, op0=MAX)
                    rd = pre.tile([P, 1], F32, tag="rd")
                    nc.vector.reciprocal(rd[:], dmax[:])
                    nc.vector.tensor_scalar(alpha_loop_sb[:, t * 9:(t + 1) * 9],
                                            psum_agg[:, :9], rd[:], None, op0=MULT)

            # ================= LAYERS =================
            for l in (1, 2, 3):
                heads, C, ROW, ASL = LHEADS[l], LC[l], LROW[l], LAS[l]
                HW = C // heads
                # ---------- projection -> ag_in[l] ----------
                with tc.tile_pool(name=f"proj{l}", bufs=2) as pj, \
                     tc.tile_pool(name=f"projp{l}", bufs=2, space="PSUM") as pjp:
                    for t in range(NT):
                        psum_x = pjp.tile([P, C], F32, tag="px")
                        psum_a = pjp.tile([P, 2 * heads], F32, tag="pa")
                        if l == 1:
                            xt = pj.tile([D_NODE, P], F32, tag="xt")
                            nc.sync.dma_start(xt[:], xT_dram[:, t * P:(t + 1) * P])
                            nc.tensor.matmul(psum_x[:], lhsT=xt[:], rhs=W1ext_sb[:, :C], start=True, stop=True)
                            nc.tensor.matmul(psum_a[:], lhsT=xt[:], rhs=W1ext_sb[:, C:C + 2 * heads], start=True, stop=True)
                        else:
                            Wsb = W2ext_sb if l == 2 else W3ext_sb
                            WR = 520 if l == 2 else 66
                            for kb in range(4):
                                psum_hT = pjp.tile([P, P], BF, tag="phT")
                                nc.tensor.transpose(psum_hT[:], h_slab[:, t * 512 + kb * 128: t * 512 + (kb + 1) * 128], ident_bf[:])
                                hT = pj.tile([P, P], BF, tag="hT")
                                nc.scalar.copy(hT[:], psum_hT[:])
                                nc.tensor.matmul(psum_x[:], lhsT=hT[:], rhs=Wsb[:, kb * WR:kb * WR + C],
                                                 start=(kb == 0), stop=(kb == 3))
                                nc.tensor.matmul(psum_a[:], lhsT=hT[:], rhs=Wsb[:, kb * WR + C:kb * WR + C + 2 * heads],
                                                 start=(kb == 0), stop=(kb == 3))
                        nc.scalar.copy(asd_own[:, t * 8:t * 8 + heads], psum_a[:, :heads])
                        nc.scalar.copy(asd_own[:, t * 8 + 4:t * 8 + 4 + heads], psum_a[:, heads:2 * heads])
                        nc.vector.tensor_copy(ad_bf[:, t * 4:t * 4 + heads], psum_a[:, heads:2 * heads])
                        row = pj.tile([P, ROW], BF, tag="row")
                        row_f32 = row[:].bitcast(F32)
                        nc.vector.tensor_copy(row_f32[:, :heads], psum_a[:, :heads])
                        nc.scalar.copy(row[:, ASL:ASL + C], psum_x[:])
                        nc.sync.dma_start(ag_in[l][t * P:(t + 1) * P, :], row[:])
                    nc.vector.tensor_tensor(
                        out=asum_own[:].rearrange("p (t k) -> p t k", k=4)[:, :, :heads],
                        in0=asd_own[:].rearrange("p (t k) -> p t k", k=8)[:, :, :heads],
                        in1=asd_own[:].rearrange("p (t k) -> p t k", k=8)[:, :, 4:4 + heads],
                        op=ADD)

                # ---------- AllGather ----------
                nc.gpsimd.collective_compute(
                    "AllGather", mybir.AluOpType.bypass, replica_groups=RG,
                    ins=[ag_in[l][:]], outs=[table[l][:]],
                )

                # ---------- main pass ----------
                NCHUNK = CPT + 1
                AW = heads * NCHUNK
                with tc.tile_pool(name=f"main{l}", bufs=2) as mn, \
                     tc.tile_pool(name=f"mainp{l}", bufs=2, space="PSUM") as mp, \
                     (tc.tile_pool(name="poolp", bufs=1, space="PSUM") if l == 3 else _nullpool()) as plp:
                    if l == 3:
                        psum_pool = plp.tile([H2, GPC], F32, tag="pool")
                    for t in range(NT):
                        gbuf = mn.tile([P, NCHUNK * ROW], BF, tag="gbuf")
                        nc.gpsimd.dma_gather(
                            gbuf[:, :CPT_A * ROW].rearrange("p (c e) -> p c e", e=ROW),
                            table[l][:], idx_sb[:, t * IDXW: t * IDXW + CA // 16],
                            CA, CA, ROW)
                        nc.gpsimd.dma_gather(
                            gbuf[:, CPT_A * ROW:CPT * ROW].rearrange("p (c e) -> p c e", e=ROW),
                            table[l][BUCKET:, :], idx_sb[:, t * IDXW + CA // 16: t * IDXW + IDXW],
                            CB, CB, ROW)
                        nc.sync.dma_start(gbuf[:, CPT * ROW:], ag_in[l][t * P:(t + 1) * P, :])
                        s_blk = mn.tile([P, SB], BF, tag="sblk")
                        nc.sync.dma_start(s_blk[:], s_dram[t])
                        st_blk = mn.tile([P, SB], BF, tag="stblk")
                        nc.sync.dma_start(st_blk[:], st_dram[t])

                        psum_za = mp.tile([P, AW + heads], F32, tag="za")
                        for c in range(CPT):
                            nc.tensor.matmul(psum_za[:, heads + c * heads: heads + (c + 1) * heads],
                                             lhsT=st_blk[:, c * P:(c + 1) * P],
                                             rhs=ad_bf[:, t * 4:t * 4 + heads],
                                             start=True, stop=True)
                        # ---- alpha assembly ----
                        t_al = mn.tile([P, AW], F32, tag="t_al")
                        gb_f32 = gbuf[:].bitcast(F32).rearrange("p (c e) -> p c e", e=ROW // 2)
                        nc.vector.tensor_tensor(
                            out=t_al[:].rearrange("p (c k) -> p c k", k=heads)[:, :CPT, :],
                            in0=gb_f32[:, :CPT, :heads],
                            in1=alpha_e_sb[:, t * CPT * 9:(t + 1) * CPT * 9].rearrange(
                                "p (c k) -> p c k", k=9)[:, :, AOFF[l]:AOFF[l] + heads],
                            op=ADD)
                        nc.vector.tensor_tensor(
                            out=t_al[:, CPT * heads:],
                            in0=asum_own[:, t * 4:t * 4 + heads],
                            in1=alpha_loop_sb[:, t * 9 + AOFF[l]: t * 9 + AOFF[l] + heads],
                            op=ADD)
                        nc.vector.tensor_tensor(out=t_al[:, :CPT * heads], in0=t_al[:, :CPT * heads],
                                                in1=psum_za[:, heads:heads + CPT * heads], op=ADD)
                        t_lr = mn.tile([P, AW], F32, tag="t_lr")
                        nc.vector.tensor_scalar(t_lr[:], t_al[:], NEG, None, op0=MULT)
                        nc.vector.tensor_tensor(out=t_al[:], in0=t_al[:], in1=t_lr[:], op=MAX)
                        p_f = mn.tile([P, AW], F32, tag="p_f")
                        nc.scalar.activation(p_f[:], t_al[:], EXP)
                        if heads == 4:
                            mview = mask4_sb[:, t * CPT * 4:(t + 1) * CPT * 4]
                        else:
                            mview = mask4_sb[:].rearrange("p (n k) -> p n k", k=4)[:, t * CPT:(t + 1) * CPT, 0:1].rearrange("p n k -> p (n k)")
                        nc.vector.tensor_tensor(out=p_f[:, :CPT * heads], in0=p_f[:, :CPT * heads],
                                                in1=mview, op=MULT)
                        p_bf = mn.tile([P, AW], BF, tag="p_bf")
                        nc.vector.tensor_copy(p_bf[:], p_f[:])
                        # ---- messages + scatter (Z fused into M as extra cols) ----
                        if l < 3:
                            psum_M1 = mp.tile([P, 260], F32, tag="M1")
                            psum_M2 = mp.tile([P, 260], F32, tag="M2")
                        else:
                            psum_M1 = mp.tile([P, C + 1], F32, tag="M1")
                        for c in range(NCHUNK):
                            Sw = s_blk[:, c * P:(c + 1) * P] if c < CPT else ident_bf[:]
                            g_xs = gbuf[:, c * ROW + ASL: c * ROW + ASL + C]
                            if l < 3:
                                m_t = mn.tile([P, 520], BF, tag="m")
                                nc.vector.tensor_tensor(
                                    out=m_t[:, :256].rearrange("p (a b) -> p a b", b=HW),
                                    in0=g_xs[:, :256].rearrange("p (a b) -> p a b", b=HW),
                                    in1=p_bf[:, c * heads:c * heads + 2].rearrange("p (a b) -> p a b", b=1).to_broadcast([P, 2, HW]),
                                    op=MULT)
                                nc.vector.tensor_copy(m_t[:, 256:260], p_bf[:, c * heads:(c + 1) * heads])
                                nc.vector.tensor_tensor(
                                    out=m_t[:, 260:516].rearrange("p (a b) -> p a b", b=HW),
                                    in0=g_xs[:, 256:512].rearrange("p (a b) -> p a b", b=HW),
                                    in1=p_bf[:, c * heads + 2:(c + 1) * heads].rearrange("p (a b) -> p a b", b=1).to_broadcast([P, 2, HW]),
                                    op=MULT)
                                nc.tensor.matmul(psum_M1[:], lhsT=Sw, rhs=m_t[:, :260],
                                                 start=(c == 0), stop=(c == NCHUNK - 1))
                                nc.tensor.matmul(psum_M2[:], lhsT=Sw, rhs=m_t[:, 260:520],
                                                 start=(c == 0), stop=(c == NCHUNK - 1))
                            else:
                                m_t = mn.tile([P, C + 1], BF, tag="m")
                                nc.vector.tensor_tensor(
                                    out=m_t[:, :C],
                                    in0=g_xs[:],
                                    in1=p_bf[:, c:c + 1].to_broadcast([P, C]),
                                    op=MULT)
                                nc.vector.tensor_copy(m_t[:, C:C + 1], p_bf[:, c:c + 1])
                                nc.tensor.matmul(psum_M1[:], lhsT=Sw, rhs=m_t[:],
                                                 start=(c == 0), stop=(c == NCHUNK - 1))
                        # ---- epilogue ----
                        if l < 3:
                            zt = mn.tile([P, heads], F32, tag="zt")
                            nc.vector.tensor_scalar(zt[:], psum_M1[:, 256:260], 1e-16, None, op0=ADD)
                            rz = mn.tile([P, heads], F32, tag="rz")
                            nc.vector.reciprocal(rz[:], zt[:])
                            ht = mn.tile([P, C], F32, tag="ht")
                            nc.vector.tensor_tensor(
                                out=ht[:, :256].rearrange("p (a b) -> p a b", b=HW),
                                in0=psum_M1[:, :256].rearrange("p (a b) -> p a b", b=HW),
                                in1=rz[:, 0:2].rearrange("p (a b) -> p a b", b=1).to_broadcast([P, 2, HW]), op=MULT)
                            nc.vector.tensor_tensor(
                                out=ht[:, 256:512].rearrange("p (a b) -> p a b", b=HW),
                                in0=psum_M2[:, :256].rearrange("p (a b) -> p a b", b=HW),
                                in1=rz[:, 2:4].rearrange("p (a b) -> p a b", b=1).to_broadcast([P, 2, HW]), op=MULT)
                            nc.vector.tensor_tensor(out=ht[:], in0=ht[:], in1=B_sb[l][:, :C], op=ADD)
                            nc.scalar.activation(h_slab[:, t * 512:(t + 1) * 512], ht[:], RELU)
                        else:
                            zt = mn.tile([P, 1], F32, tag="zt")
                            nc.vector.tensor_scalar(zt[:], psum_M1[:, C:C + 1], 1e-16, None, op0=ADD)
                            rz = mn.tile([P, 1], F32, tag="rz")
                            nc.vector.reciprocal(rz[:], zt[:])
                            ht = mn.tile([P, C], F32, tag="ht")
                            nc.vector.tensor_scalar(ht[:], psum_M1[:, :C], rz[:], None, op0=MULT)
                            nc.vector.tensor_tensor(out=ht[:], in0=ht[:], in1=B_sb[3][:, :C], op=ADD)
                            h3 = mn.tile([P, C], F32, tag="h3")
                            nc.scalar.activation(h3[:], ht[:], RELU)
                            Sp = mn.tile([P, GPC], F32, tag="Sp")
                            nc.vector.tensor_scalar(Sp[:], iota16[:], gl_node_sb[:, t:t + 1], None, op0=EQ)
                            nc.vector.tensor_tensor(out=Sp[:], in0=Sp[:], in1=inv_cnt_sb[:], op=MULT)
                            nc.tensor.matmul(psum_pool[:], lhsT=h3[:], rhs=Sp[:],
                                             start=(t == 0), stop=(t == NT - 1))

            # ================= FINAL: og norm + FFN + softmax =================
            with tc.tile_pool(name="fin", bufs=1) as fin, \
                 tc.tile_pool(name="finp", bufs=1, space="PSUM") as fnp:
                sq = fin.tile([GPC, EDGE_DIM - 1], F32, tag="sq")
                nc.scalar.activation(sq[:], og_raw[:], SQUARE)
                ss = fin.tile([GPC, 1], F32, tag="ss")
                nc.vector.tensor_reduce(out=ss[:], in_=sq[:], axis=mybir.AxisListType.X, op=ADD)
                nc.vector.tensor_scalar(ss[:], ss[:], 1e-24, None, op0=MAX)
                iss = fin.tile([GPC, 1], F32, tag="iss")
                nc.vector.reciprocal(iss[:], ss[:])
                rs = fin.tile([GPC, 1], F32, tag="rs")
                nc.scalar.activation(rs[:], iss[:], SQRT)
                ogn = fin.tile([GPC, EDGE_DIM - 1], F32, tag="ogn")
                nc.vector.tensor_scalar(ogn[:], og_raw[:], rs[:], None, op0=MULT)
                psum_ogT = fnp.tile([EDGE_DIM - 1, GPC], F32, tag="ogT")
                nc.tensor.transpose(psum_ogT[:], ogn[:], ident_f32[:GPC, :GPC])
                ogT = fin.tile([EDGE_DIM - 1, GPC], F32, tag="ogTs")
                nc.scalar.copy(ogT[:], psum_ogT[:])
                pooledT = fin.tile([H2, GPC], F32, tag="pooledT")
                nc.scalar.copy(pooledT[:], psum_pool[:])
                psum_z1 = fnp.tile([67, GPC], F32, tag="z1")
                nc.tensor.matmul(psum_z1[:], lhsT=Wf1a_sb[:], rhs=pooledT[:], start=True, stop=False)
                nc.tensor.matmul(psum_z1[:], lhsT=Wf1b_sb[:], rhs=ogT[:], start=False, stop=True)
                z1 = fin.tile([67, GPC], F32, tag="z1s")
                nc.scalar.activation(z1[:], psum_z1[:], RELU, bias=bf1c_sb[:])
                psum_z2 = fnp.tile([NCLS, GPC], F32, tag="z2")
                nc.tensor.matmul(psum_z2[:], lhsT=Wf2_sb[:], rhs=z1[:], start=True, stop=True)
                z2b = fin.tile([NCLS, GPC], F32, tag="z2b")
                nc.scalar.activation(z2b[:], psum_z2[:], IDENT, bias=bf2c_sb[:])
                psum_z2T = fnp.tile([GPC, NCLS], F32, tag="z2T")
                nc.tensor.transpose(psum_z2T[:], z2b[:], ident_f32[:NCLS, :NCLS])
                e2 = fin.tile([GPC, NCLS], F32, tag="e2")
                nc.scalar.activation(e2[:], psum_z2T[:], EXP)
                s2 = fin.tile([GPC, 1], F32, tag="s2")
                nc.vector.tensor_reduce(out=s2[:], in_=e2[:], axis=mybir.AxisListType.X, op=ADD)
                r2 = fin.tile([GPC, 1], F32, tag="r2")
                nc.vector.reciprocal(r2[:], s2[:])
                o2 = fin.tile([GPC, NCLS], F32, tag="o2")
                nc.vector.tensor_scalar(o2[:], e2[:], r2[:], None, op0=MULT)
                nc.sync.dma_start(out_dram[:], o2[:])

    nc.compile()
    return nc


import contextlib


@contextlib.contextmanager
def _nullpool():
    yield None


def kernel(**inputs) -> np.ndarray:
    dims, const, percore, node_start = host_prep(inputs)
    nc = build_program(dims, const)
    in_maps = []
    for k in range(NCORES):
        m = {name: np.ascontiguousarray(arr) for name, arr in const.items()}
        m.update(
            eaT_stream=percore["eaT_stream"][k],
            ea_og=percore["ea_og"][k],
            gl_og=percore["gl_og"][k],
            idx_w=percore["idx_w"][k],
            mask4=percore["mask4"][k],
            dstl=percore["dstl"][k],
            x_T=percore["x_T"][k],
            gl_node=percore["gl_node"][k],
            inv_cnt=percore["inv_cnt"][k],
        )
        in_maps.append(m)
    trace = bool(int(os.environ.get("BASS_KERNEL_TRACE", "0")))
    if trace:
        try:
            import sys as _sys, types as _types
            if "antenv.axon_hooks" not in _sys.modules:
                _m = _types.ModuleType("antenv.axon_hooks")
                _h = [None]

                def _get():
                    if _h[0] is None:
                        from trn_agent_boot.trn_boot import _ntff_profile_via_ctypes
                        _h[0] = _ntff_profile_via_ctypes("/opt/axon/libaxon_pjrt.so")
                    return _h[0]

                _m.get_axon_ntff_profile_hook = _get
                _m.set_axon_ntff_profile_hook = lambda h: _h.__setitem__(0, h)
                _sys.modules["antenv.axon_hooks"] = _m
        except Exception:
            trace = False
    res = run_bass_kernel_spmd(nc, in_maps, core_ids=list(range(NCORES)), trace=trace)
    if trace and res.exec_time_ns is not None:
        print(f"HW exec time: {res.exec_time_ns} ns")
    out = np.zeros((G, NCLS), np.float32)
    for k in range(NCORES):
        out[k * GPC:(k + 1) * GPC] = np.asarray(res.results[k]["out_gc"], np.float32)
    return out


# revision 14
# speedup vs baseline: 1.2301x; 1.0083x over previous
"""Trainium2 Bass kernel for nn_GAT_mlp_fed_1gram (3-layer GAT + 1-gram + FFN).

Self-contained: host-side numpy prep (sharding/sorting/index build + small-weight
folding) + an 8-core SPMD Bass/Tile program (graph-parallel slabs, dma_gather of
projected node features from an AllGathered table, one-hot-matmul segment
softmax/scatter, local pooling + FFN), assembled back to the full [128, 2] output.

Algorithm notes (validated against the reference in numpy):
  - (ee*a_e).sum(-1) folds to edge_attr @ (We . a_e)  -> [72, heads] per layer
  - (xs*a_s).sum(-1) folds into the projection: h @ [W | W.As | W.Ad]
  - segment softmax without max-subtraction (alpha is O(1)), normalization by
    post-division:  out = (sum_e e^a * xs_src) / (sum_e e^a + 1e-16)
  - loop_attr @ Wae == segment_mean(edge alphas): self-loop alphas come from a
    segment-mean of the per-edge folded alphas (matmul commutes with seg-sum)
  - self-loops (edge_attr fill 'mean') handled as one identity-chunk per tile
  - one-hot scatter matrices S / S^T are built once in the prepass and cached
    in DRAM; all three layers stream them back
"""
import os
import numpy as np
import ml_dtypes

import concourse.bacc as bacc
import concourse.mybir as mybir
import concourse.tile as tile
from concourse.bass_utils import run_bass_kernel_spmd
from concourse.library_config import mlp as _mlp_lib

BF16 = ml_dtypes.bfloat16
F32 = mybir.dt.float32
BF = mybir.dt.bfloat16
I16 = mybir.dt.int16

N, E, G = 50000, 400000, 128
D_NODE, EDGE_DIM, HEADS = 64, 72, 4
H0, H1, H2 = 128, 128, 64
NCLS = 2
NEG = 0.2
NCORES = 8
GPC = G // NCORES
P = 128
BUCKET = 32768
AOFF = {1: 0, 2: 4, 3: 8}
LHEADS = {1: HEADS, 2: HEADS, 3: 1}
LC = {1: HEADS * H0, 2: HEADS * H1, 3: H2}
LROW = {1: 640, 2: 640, 3: 128}   # bf16 slots per table row (stride, 256B mult)
LAS = {1: 8, 2: 8, 3: 2}          # leading bf16 slots holding fp32 asrc
EXP = mybir.ActivationFunctionType.Exp
RELU = mybir.ActivationFunctionType.Relu
COPY = mybir.ActivationFunctionType.Copy
SQUARE = mybir.ActivationFunctionType.Square
SQRT = mybir.ActivationFunctionType.Sqrt
IDENT = mybir.ActivationFunctionType.Identity
EQ = mybir.AluOpType.is_equal
MULT = mybir.AluOpType.mult
ADD = mybir.AluOpType.add
MAX = mybir.AluOpType.max


def _wrap16(idx):
    """dma_gather idx layout: idx i -> [i%16, i//16], replicated to 128 partitions."""
    n = len(idx)
    assert n % 16 == 0
    w = np.zeros((16, n // 16), np.int16)
    w[np.arange(n) % 16, np.arange(n) // 16] = idx
    return np.tile(w, (8, 1))


def host_prep(inputs):
    x = np.ascontiguousarray(np.asarray(inputs["x"], np.float32))
    ei = np.asarray(inputs["edge_index"])
    ea = np.ascontiguousarray(np.asarray(inputs["edge_attr"], np.float32))
    batch = np.asarray(inputs["batch"]).astype(np.int64)
    src, dst = ei[0].astype(np.int64), ei[1].astype(np.int64)

    node_start = np.searchsorted(batch, np.arange(0, G + 1, GPC))
    NT = int(np.ceil(np.diff(node_start).max() / P))
    NMAX = NT * P
    core_of_node = np.searchsorted(node_start[1:], np.arange(N), side="right")
    local_of_node = np.arange(N) - node_start[core_of_node]
    table_row = core_of_node * NMAX + local_of_node

    e_core = core_of_node[dst]
    per_core = []
    CA_need = CB_need = 0
    for k in range(NCORES):
        sel = np.nonzero(e_core == k)[0]
        d_loc = local_of_node[dst[sel]]
        order = np.argsort(d_loc, kind="stable")
        sel, d_loc = sel[order], d_loc[order]
        s_row = table_row[src[sel]]
        per_core.append((sel, d_loc, s_row))
        t_of = d_loc // P
        for t in range(NT):
            m = t_of == t
            ca = int((s_row[m] < BUCKET).sum())
            CA_need = max(CA_need, ca)
            CB_need = max(CB_need, int(m.sum()) - ca)
    CPT_A = max(1, int(np.ceil(CA_need / P)))
    CPT_B = max(1, int(np.ceil(CB_need / P)))
    CPT = CPT_A + CPT_B
    CA, CB = CPT_A * P, CPT_B * P

    idx_w = np.zeros((NCORES, NT, 128, (CA + CB) // 16), np.int16)
    # pads get dstl=127.5: the one-hot S never matches them, so they
    # contribute nothing to any scatter matmul (no mask needed anywhere)
    dstl = np.full((NCORES, 128, NT * CPT), 127.5, np.float32)
    # eaT layout: per tile [73, CPT*128]; row 72 = all-ones indicator
    eaT_stream = np.zeros((NCORES, NT, EDGE_DIM + 1, CPT * P), BF16)
    eaT_stream[:, :, EDGE_DIM, :] = 1.0
    og_core = (batch[src] // GPC).astype(np.int64)
    NOG = max(int((og_core == k).sum()) for k in range(NCORES))
    NOG = int(np.ceil(NOG / (4 * P))) * 4 * P
    ea_og = np.zeros((NCORES, NOG // P, P, EDGE_DIM), BF16)
    gl_og = np.full((NCORES, 128, NOG // P), 200.0, np.float32)

    ea_bf = ea.astype(BF16)
    for k in range(NCORES):
        sel, d_loc, s_row = per_core[k]
        t_of = d_loc // P
        for t in range(NT):
            m = np.nonzero(t_of == t)[0]
            sa = m[s_row[m] < BUCKET]
            sb_ = m[s_row[m] >= BUCKET]
            ia = np.zeros(CA, np.int16)
            ib = np.zeros(CB, np.int16)
            ia[:len(sa)] = s_row[sa].astype(np.int16)
            ib[:len(sb_)] = (s_row[sb_] - BUCKET).astype(np.int16)
            idx_w[k, t] = np.concatenate([_wrap16(ia), _wrap16(ib)], 1)
            for c_off, rows in ((0, sa), (CA, sb_)):
                nn_ = len(rows)
                j = np.arange(nn_)
                cols = (t * CPT * P + c_off + j)
                dstl[k, (cols % P), (cols // P)] = (d_loc[rows] - t * P).astype(np.float32)
                eaT_stream[k, t, :EDGE_DIM, c_off + j] = ea_bf[sel[rows]]
        m = np.nonzero(og_core == k)[0]
        j = np.arange(len(m))
        ea_og[k, j // P, j % P] = ea_bf[m]
        gl_og[k, (j % P), (j // P)] = (batch[src[m]] - k * GPC).astype(np.float32)

    def fold(W, a_s, a_d, heads):
        Wr = np.asarray(W, np.float32).reshape(W.shape[0], heads, -1)
        return np.concatenate([np.einsum("dhc,hc->dh", Wr, np.asarray(a_s, np.float32)),
                               np.einsum("dhc,hc->dh", Wr, np.asarray(a_d, np.float32))], 1)

    W_ext = {
        1: np.concatenate([np.asarray(inputs["W1"], np.float32),
                           fold(inputs["W1"], inputs["as1"], inputs["ad1"], HEADS)], 1),
        2: np.concatenate([np.asarray(inputs["W2"], np.float32),
                           fold(inputs["W2"], inputs["as2"], inputs["ad2"], HEADS)], 1),
        3: np.concatenate([np.asarray(inputs["W3"], np.float32),
                           fold(inputs["W3"], inputs["as3"], inputs["ad3"], 1)], 1),
    }
    Wae0 = np.concatenate([
        np.einsum("dhc,hc->dh", np.asarray(inputs["We1"], np.float32).reshape(EDGE_DIM, HEADS, H0), np.asarray(inputs["ae1"], np.float32)),
        np.einsum("dhc,hc->dh", np.asarray(inputs["We2"], np.float32).reshape(EDGE_DIM, HEADS, H1), np.asarray(inputs["ae2"], np.float32)),
        np.einsum("dhc,hc->dh", np.asarray(inputs["We3"], np.float32).reshape(EDGE_DIM, 1, H2), np.asarray(inputs["ae3"], np.float32)),
    ], 1)  # [72, 9]
    # [73, 10]: rows 0..71 = folded edge-alpha weights, col 9 picks the
    # indicator row -> per-edge constant 1 (deg accumulates via S)
    Wae = np.zeros((EDGE_DIM + 1, 10), np.float32)
    Wae[:EDGE_DIM, :9] = Wae0
    Wae[EDGE_DIM, 9] = 1.0

    x_T = np.zeros((NCORES, D_NODE, NMAX), np.float32)
    gl_node = np.full((NCORES, 128, NT), 200.0, np.float32)
    inv_cnt = np.zeros((NCORES, 128, GPC), np.float32)
    for k in range(NCORES):
        n0, n1 = node_start[k], node_start[k + 1]
        x_T[k, :, :n1 - n0] = x[n0:n1].T
        loc = np.arange(n1 - n0)
        gl_node[k, loc % P, loc // P] = (batch[n0:n1] - k * GPC).astype(np.float32)
        cnt = np.bincount(batch[n0:n1] - k * GPC, minlength=GPC).astype(np.float32)
        inv_cnt[k] = np.tile(1.0 / np.maximum(cnt, 1.0), (P, 1))

    const = dict(
        iota_row=np.tile(np.arange(P, dtype=np.float32), (P, 1)),
        iota16=np.tile(np.arange(GPC, dtype=np.float32), (P, 1)),
        ident_bf=np.eye(P, dtype=np.float32).astype(BF16),
        ident_f32=np.eye(P, dtype=np.float32),
        Wae=Wae.astype(BF16),
        W1ext=W_ext[1].astype(np.float32),
        W2ext=W_ext[2].reshape(4, 128, 520).transpose(1, 0, 2).reshape(128, 4 * 520).astype(BF16),
        W3ext=W_ext[3].reshape(4, 128, 66).transpose(1, 0, 2).reshape(128, 4 * 66).astype(BF16),
        B1=np.tile(np.asarray(inputs["b1"], np.float32), (P, 1)),
        B2=np.tile(np.asarray(inputs["b2"], np.float32), (P, 1)),
        B3=np.tile(np.asarray(inputs["b3"], np.float32), (P, 1)),
        Wf1a=np.asarray(inputs["Wf1"], np.float32)[:H2],
        Wf1b=np.asarray(inputs["Wf1"], np.float32)[H2:],
        Wf2=np.asarray(inputs["Wf2"], np.float32),
        bf1c=np.asarray(inputs["bf1"], np.float32)[:, None],
        bf2c=np.asarray(inputs["bf2"], np.float32)[:, None],
    )
    dims = dict(NT=NT, NMAX=NMAX, CPT_A=CPT_A, CPT_B=CPT_B, CPT=CPT, CA=CA, CB=CB, NOG=NOG)
    percore = dict(idx_w=idx_w, dstl=dstl, eaT_stream=eaT_stream,
                   ea_og=ea_og, gl_og=gl_og, x_T=x_T, gl_node=gl_node, inv_cnt=inv_cnt)
    return dims, const, percore, node_start


def build_program(dims, const):
    NT, NMAX = dims["NT"], dims["NMAX"]
    CPT_A, CPT_B, CPT = dims["CPT_A"], dims["CPT_B"], dims["CPT"]
    CA, CB = dims["CA"], dims["CB"]
    NOG = dims["NOG"]
    NOGC = NOG // P
    IDXW = (CA + CB) // 16
    SB = CPT * P                       # S-block width per tile

    nc = bacc.Bacc("TRN2", target_bir_lowering=False, debug=False, num_devices=NCORES)

    din = {}
    def dram_in(name, shape, dt=F32):
        din[name] = nc.dram_tensor(name, list(shape), dt, kind="ExternalInput")
        return din[name]

    eaT_dram = dram_in("eaT_stream", [NT, EDGE_DIM + 1, CPT * P], BF)
    ea_og_dram = dram_in("ea_og", [NOGC, P, EDGE_DIM], BF)
    gl_og_dram = dram_in("gl_og", [P, NOGC])
    idx_dram = dram_in("idx_w", [NT, P, IDXW], I16)
    dstl_dram = dram_in("dstl", [P, NT * CPT])
    xT_dram = dram_in("x_T", [D_NODE, NMAX])
    gl_node_dram = dram_in("gl_node", [P, NT])
    inv_cnt_dram = dram_in("inv_cnt", [P, GPC])
    for cname, arr in const.items():
        dram_in(cname, arr.shape, BF if arr.dtype == BF16 else F32)

    out_dram = nc.dram_tensor("out_gc", [GPC, NCLS], F32, kind="ExternalOutput")

    ag_in = {l: nc.dram_tensor(f"ag_in{l}", [NMAX, LROW[l]], BF, kind="Internal")
             for l in (1, 2, 3)}
    table = {l: nc.dram_tensor(f"table{l}", [NCORES * NMAX, LROW[l]], BF,
                               kind="Internal", addr_space="Shared") for l in (1, 2, 3)}
    s_dram = nc.dram_tensor("s_blocks", [NT, P, SB], BF, kind="Internal")
    st_dram = nc.dram_tensor("st_blocks", [NT, P, SB], BF, kind="Internal")

    RG = [list(range(NCORES))]

    with tile.TileContext(nc) as tc:
        nc.gpsimd.load_library(_mlp_lib)
        import contextlib
        ctx = contextlib.ExitStack()
        with ctx:
            persist = ctx.enter_context(tc.tile_pool(name="persist", bufs=1))

            def pload(name, shape=None, dt=F32):
                t = persist.tile(list(shape if shape is not None else const[name].shape), dt, tag=name)
                nc.sync.dma_start(t[:], din[name][:])
                return t

            iota_row = pload("iota_row")
            iota16 = pload("iota16")
            ident_bf = pload("ident_bf", dt=BF)
            ident_f32 = pload("ident_f32")
            Wae_sb = pload("Wae", dt=BF)
            W1ext_sb = pload("W1ext")
            W2ext_sb = pload("W2ext", dt=BF)
            W3ext_sb = pload("W3ext", dt=BF)
            B_sb = {1: pload("B1"), 2: pload("B2"), 3: pload("B3")}
            Wf1a_sb = pload("Wf1a"); Wf1b_sb = pload("Wf1b"); Wf2_sb = pload("Wf2")
            bf1c_sb = pload("bf1c"); bf2c_sb = pload("bf2c")
            dstl_sb = persist.tile([P, NT * CPT], F32, tag="dstl")
            nc.sync.dma_start(dstl_sb[:], dstl_dram[:])
            idx_sb = persist.tile([P, NT * IDXW], I16, tag="idx")
            nc.sync.dma_start(idx_sb[:].rearrange("p (t k) -> p t k", t=NT),
                              idx_dram[:].rearrange("t p k -> p t k"))
            gl_node_sb = persist.tile([P, NT], F32, tag="gl_node")
            nc.sync.dma_start(gl_node_sb[:], gl_node_dram[:])
            inv_cnt_sb = persist.tile([P, GPC], F32, tag="inv_cnt")
            nc.sync.dma_start(inv_cnt_sb[:], inv_cnt_dram[:])
            gl_og_sb = persist.tile([P, NOGC], F32, tag="gl_og")
            nc.sync.dma_start(gl_og_sb[:], gl_og_dram[:])

            alpha_e_sb = persist.tile([P, NT * CPT * 9], F32, tag="alpha_e")
            alpha_loop_sb = persist.tile([P, NT * 9], F32, tag="alpha_loop")
            asd_own = persist.tile([P, NT * 8], F32, tag="asd_own")
            asum_own = persist.tile([P, NT * 4], F32, tag="asum_own")
            ad_bf = persist.tile([P, NT * 4], BF, tag="ad_bf")
            h_slab = persist.tile([P, NT * 512], BF, tag="h_slab")
            og_raw = persist.tile([GPC, EDGE_DIM - 1], F32, tag="og_raw")

            # ================= PREPASS =================
            with tc.tile_pool(name="pre", bufs=3) as pre, \
                 tc.tile_pool(name="sblk", bufs=2) as sblk, \
                 tc.tile_pool(name="prepsum", bufs=2, space="PSUM") as pps, \
                 tc.tile_pool(name="ogpsum", bufs=1, space="PSUM") as ogp:
                psum_og = ogp.tile([GPC, EDGE_DIM - 1], F32, tag="og")
                OGB = 4
                assert NOGC % OGB == 0
                for ob in range(NOGC // OGB):
                    eo = pre.tile([P, OGB, EDGE_DIM], BF, tag="eo")
                    nc.sync.dma_start(eo[:], ea_og_dram[:].rearrange("a p d -> p a d")[:, ob * OGB:(ob + 1) * OGB, :])
                    for a in range(OGB):
                        oc = ob * OGB + a
                        Sog = pre.tile([P, GPC], BF, tag="sog")
                        nc.vector.tensor_scalar(Sog[:], iota16[:], gl_og_sb[:, oc:oc + 1], None, op0=EQ)
                        nc.tensor.matmul(psum_og[:], lhsT=Sog[:], rhs=eo[:, a, :EDGE_DIM - 1],
                                         start=(oc == 0), stop=(oc == NOGC - 1))
                nc.scalar.copy(og_raw[:], psum_og[:])

                for t in range(NT):
                    # --- build S block + S^T block, cache to DRAM ---
                    s_blk = sblk.tile([P, SB], BF, tag="s")
                    st_blk = sblk.tile([P, SB], BF, tag="st")
                    for c in range(CPT):
                        tcn = t * CPT + c
                        nc.vector.tensor_scalar(s_blk[:, c * P:(c + 1) * P], iota_row[:],
                                                dstl_sb[:, tcn:tcn + 1], None, op0=EQ)
                        psum_ST = pps.tile([P, P], BF, tag="stp")
                        nc.tensor.transpose(psum_ST[:], s_blk[:, c * P:(c + 1) * P], ident_bf[:])
                        nc.scalar.copy(st_blk[:, c * P:(c + 1) * P], psum_ST[:])
                    nc.sync.dma_start(s_dram[t], s_blk[:])
                    nc.sync.dma_start(st_dram[t], st_blk[:])
                    # --- per-chunk folded edge alphas + segment-mean for loops ---
                    psum_agg = pps.tile([P, 10], F32, tag="agg")
                    eaT = pre.tile([EDGE_DIM + 1, CPT * P], BF, tag="eaT")
                    nc.sync.dma_start(eaT[:], eaT_dram[t])
                    for c in range(CPT):
                        tcn = t * CPT + c
                        psum_ae = pps.tile([P, 10], F32, tag="ae")
                        nc.tensor.matmul(psum_ae[:], lhsT=eaT[:, c * P:(c + 1) * P],
                                         rhs=Wae_sb[:], start=True, stop=True)
                        nc.scalar.copy(alpha_e_sb[:, tcn * 9:(tcn + 1) * 9], psum_ae[:, :9])
                        aggrhs = pre.tile([P, 10], BF, tag="aggrhs")
                        nc.scalar.copy(aggrhs[:], psum_ae[:])
                        nc.tensor.matmul(psum_agg[:], lhsT=s_blk[:, c * P:(c + 1) * P],
                                         rhs=aggrhs[:], start=(c == 0), stop=(c == CPT - 1))
                    dmax = pre.tile([P, 1], F32, tag="dmax")
                    nc.vector.tensor_scalar(dmax[:], psum_agg[:, 9:10], 1.0, None, op0=MAX)
                    rd = pre.tile([P, 1], F32, tag="rd")
                    nc.vector.reciprocal(rd[:], dmax[:])
                    nc.vector.tensor_scalar(alpha_loop_sb[:, t * 9:(t + 1) * 9],
                                            psum_agg[:, :9], rd[:], None, op0=MULT)

            # ================= LAYERS =================
            for l in (1, 2, 3):
                heads, C, ROW, ASL = LHEADS[l], LC[l], LROW[l], LAS[l]
                HW = C // heads
                # ---------- projection -> ag_in[l] ----------
                with tc.tile_pool(name=f"proj{l}", bufs=2) as pj, \
                     tc.tile_pool(name=f"projp{l}", bufs=2, space="PSUM") as pjp:
                    for t in range(NT):
                        psum_x = pjp.tile([P, C], F32, tag="px")
                        psum_a = pjp.tile([P, 2 * heads], F32, tag="pa")
                        if l == 1:
                            xt = pj.tile([D_NODE, P], F32, tag="xt")
                            nc.sync.dma_start(xt[:], xT_dram[:, t * P:(t + 1) * P])
                            nc.tensor.matmul(psum_x[:], lhsT=xt[:], rhs=W1ext_sb[:, :C], start=True, stop=True)
                            nc.tensor.matmul(psum_a[:], lhsT=xt[:], rhs=W1ext_sb[:, C:C + 2 * heads], start=True, stop=True)
                        else:
                            Wsb = W2ext_sb if l == 2 else W3ext_sb
                            WR = 520 if l == 2 else 66
                            for kb in range(4):
                                psum_hT = pjp.tile([P, P], BF, tag="phT")
                                nc.tensor.transpose(psum_hT[:], h_slab[:, t * 512 + kb * 128: t * 512 + (kb + 1) * 128], ident_bf[:])
                                hT = pj.tile([P, P], BF, tag="hT")
                                nc.scalar.copy(hT[:], psum_hT[:])
                                nc.tensor.matmul(psum_x[:], lhsT=hT[:], rhs=Wsb[:, kb * WR:kb * WR + C],
                                                 start=(kb == 0), stop=(kb == 3))
                                nc.tensor.matmul(psum_a[:], lhsT=hT[:], rhs=Wsb[:, kb * WR + C:kb * WR + C + 2 * heads],
                                                 start=(kb == 0), stop=(kb == 3))
                        nc.scalar.copy(asd_own[:, t * 8:t * 8 + heads], psum_a[:, :heads])
                        nc.scalar.copy(asd_own[:, t * 8 + 4:t * 8 + 4 + heads], psum_a[:, heads:2 * heads])
                        nc.vector.tensor_copy(ad_bf[:, t * 4:t * 4 + heads], psum_a[:, heads:2 * heads])
                        row = pj.tile([P, ROW], BF, tag="row")
                        row_f32 = row[:].bitcast(F32)
                        nc.vector.tensor_copy(row_f32[:, :heads], psum_a[:, :heads])
                        nc.scalar.copy(row[:, ASL:ASL + C], psum_x[:])
                        nc.sync.dma_start(ag_in[l][t * P:(t + 1) * P, :], row[:])
                    nc.vector.tensor_tensor(
                        out=asum_own[:].rearrange("p (t k) -> p t k", k=4)[:, :, :heads],
                        in0=asd_own[:].rearrange("p (t k) -> p t k", k=8)[:, :, :heads],
                        in1=asd_own[:].rearrange("p (t k) -> p t k", k=8)[:, :, 4:4 + heads],
                        op=ADD)

                # ---------- AllGather ----------
                nc.gpsimd.collective_compute(
                    "AllGather", mybir.AluOpType.bypass, replica_groups=RG,
                    ins=[ag_in[l][:]], outs=[table[l][:]],
                )

                # ---------- main pass ----------
                NCHUNK = CPT + 1
                AW = heads * NCHUNK
                with tc.tile_pool(name=f"main{l}", bufs=2) as mn, \
                     tc.tile_pool(name=f"mainp{l}", bufs=2, space="PSUM") as mp, \
                     (tc.tile_pool(name="poolp", bufs=1, space="PSUM") if l == 3 else _nullpool()) as plp:
                    if l == 3:
                        psum_pool = plp.tile([H2, GPC], F32, tag="pool")
                    for t in range(NT):
                        gbuf = mn.tile([P, NCHUNK * ROW], BF, tag="gbuf")
                        nc.gpsimd.dma_gather(
                            gbuf[:, :CPT_A * ROW].rearrange("p (c e) -> p c e", e=ROW),
                            table[l][:], idx_sb[:, t * IDXW: t * IDXW + CA // 16],
                            CA, CA, ROW)
                        nc.gpsimd.dma_gather(
                            gbuf[:, CPT_A * ROW:CPT * ROW].rearrange("p (c e) -> p c e", e=ROW),
                            table[l][BUCKET:, :], idx_sb[:, t * IDXW + CA // 16: t * IDXW + IDXW],
                            CB, CB, ROW)
                        nc.sync.dma_start(gbuf[:, CPT * ROW:], ag_in[l][t * P:(t + 1) * P, :])
                        s_blk = mn.tile([P, SB], BF, tag="sblk")
                        nc.sync.dma_start(s_blk[:], s_dram[t])
                        st_blk = mn.tile([P, SB], BF, tag="stblk")
                        nc.sync.dma_start(st_blk[:], st_dram[t])

                        psum_za = mp.tile([P, AW + heads], F32, tag="za")
                        for c in range(CPT):
                            nc.tensor.matmul(psum_za[:, heads + c * heads: heads + (c + 1) * heads],
                                             lhsT=st_blk[:, c * P:(c + 1) * P],
                                             rhs=ad_bf[:, t * 4:t * 4 + heads],
                                             start=True, stop=True)
                        # ---- alpha assembly ----
                        t_al = mn.tile([P, AW], F32, tag="t_al")
                        gb_f32 = gbuf[:].bitcast(F32).rearrange("p (c e) -> p c e", e=ROW // 2)
                        nc.vector.tensor_tensor(
                            out=t_al[:].rearrange("p (c k) -> p c k", k=heads)[:, :CPT, :],
                            in0=gb_f32[:, :CPT, :heads],
                            in1=alpha_e_sb[:, t * CPT * 9:(t + 1) * CPT * 9].rearrange(
                                "p (c k) -> p c k", k=9)[:, :, AOFF[l]:AOFF[l] + heads],
                            op=ADD)
                        nc.vector.tensor_tensor(
                            out=t_al[:, CPT * heads:],
                            in0=asum_own[:, t * 4:t * 4 + heads],
                            in1=alpha_loop_sb[:, t * 9 + AOFF[l]: t * 9 + AOFF[l] + heads],
                            op=ADD)
                        nc.vector.tensor_tensor(out=t_al[:, :CPT * heads], in0=t_al[:, :CPT * heads],
                                                in1=psum_za[:, heads:heads + CPT * heads], op=ADD)
                        t_lr = mn.tile([P, AW], F32, tag="t_lr")
                        nc.scalar.activation(t_lr[:], t_al[:], mybir.ActivationFunctionType.Lrelu, alpha=NEG)
                        p_bf = mn.tile([P, AW], BF, tag="p_bf")
                        nc.scalar.activation(p_bf[:], t_lr[:], EXP)
                        # ---- messages + scatter (Z fused into M as extra cols) ----
                        if l < 3:
                            psum_M1 = mp.tile([P, 256], F32, tag="M1")
                            psum_M2 = mp.tile([P, 260], F32, tag="M2")
                        else:
                            psum_M1 = mp.tile([P, C + 1], F32, tag="M1")
                        for c in range(NCHUNK):
                            Sw = s_blk[:, c * P:(c + 1) * P] if c < CPT else ident_bf[:]
                            g_xs = gbuf[:, c * ROW + ASL: c * ROW + ASL + C]
                            if l < 3:
                                m_t = mn.tile([P, 516], BF, tag="m")
                                nc.vector.tensor_tensor(
                                    out=m_t[:, :512].rearrange("p (a b) -> p a b", b=HW),
                                    in0=g_xs[:].rearrange("p (a b) -> p a b", b=HW),
                                    in1=p_bf[:, c * heads:(c + 1) * heads].rearrange("p (a b) -> p a b", b=1).to_broadcast([P, heads, HW]),
                                    op=MULT)
                                nc.scalar.copy(m_t[:, 512:516], p_bf[:, c * heads:(c + 1) * heads])
                                nc.tensor.matmul(psum_M1[:], lhsT=Sw, rhs=m_t[:, :256],
                                                 start=(c == 0), stop=(c == NCHUNK - 1))
                                nc.tensor.matmul(psum_M2[:], lhsT=Sw, rhs=m_t[:, 256:516],
                                                 start=(c == 0), stop=(c == NCHUNK - 1))
                            else:
                                m_t = mn.tile([P, C + 1], BF, tag="m")
                                nc.vector.tensor_tensor(
                                    out=m_t[:, :C],
                                    in0=g_xs[:],
                                    in1=p_bf[:, c:c + 1].to_broadcast([P, C]),
                                    op=MULT)
                                nc.scalar.copy(m_t[:, C:C + 1], p_bf[:, c:c + 1])
                                nc.tensor.matmul(psum_M1[:], lhsT=Sw, rhs=m_t[:],
                                                 start=(c == 0), stop=(c == NCHUNK - 1))
                        # ---- epilogue ----
                        if l < 3:
                            zt = mn.tile([P, heads], F32, tag="zt")
                            nc.vector.tensor_scalar(zt[:], psum_M2[:, 256:260], 1e-16, None, op0=ADD)
                            rz = mn.tile([P, heads], F32, tag="rz")
                            nc.vector.reciprocal(rz[:], zt[:])
                            ht = mn.tile([P, C], F32, tag="ht")
                            nc.vector.tensor_tensor(
                                out=ht[:, :256].rearrange("p (a b) -> p a b", b=HW),
                                in0=psum_M1[:, :256].rearrange("p (a b) -> p a b", b=HW),
                                in1=rz[:, 0:2].rearrange("p (a b) -> p a b", b=1).to_broadcast([P, 2, HW]), op=MULT)
                            nc.vector.tensor_tensor(
                                out=ht[:, 256:512].rearrange("p (a b) -> p a b", b=HW),
                                in0=psum_M2[:, :256].rearrange("p (a b) -> p a b", b=HW),
                                in1=rz[:, 2:4].rearrange("p (a b) -> p a b", b=1).to_broadcast([P, 2, HW]), op=MULT)
                            nc.vector.tensor_tensor(out=ht[:], in0=ht[:], in1=B_sb[l][:, :C], op=ADD)
                            nc.scalar.activation(h_slab[:, t * 512:(t + 1) * 512], ht[:], RELU)
                        else:
                            zt = mn.tile([P, 1], F32, tag="zt")
                            nc.vector.tensor_scalar(zt[:], psum_M1[:, C:C + 1], 1e-16, None, op0=ADD)
                            rz = mn.tile([P, 1], F32, tag="rz")
                            nc.vector.reciprocal(rz[:], zt[:])
                            ht = mn.tile([P, C], F32, tag="ht")
                            nc.vector.tensor_scalar(ht[:], psum_M1[:, :C], rz[:], None, op0=MULT)
                            nc.vector.tensor_tensor(out=ht[:], in0=ht[:], in1=B_sb[3][:, :C], op=ADD)
                            h3 = mn.tile([P, C], F32, tag="h3")
                            nc.scalar.activation(h3[:], ht[:], RELU)
                            Sp = mn.tile([P, GPC], F32, tag="Sp")
                            nc.vector.tensor_scalar(Sp[:], iota16[:], gl_node_sb[:, t:t + 1], None, op0=EQ)
                            nc.vector.tensor_tensor(out=Sp[:], in0=Sp[:], in1=inv_cnt_sb[:], op=MULT)
                            nc.tensor.matmul(psum_pool[:], lhsT=h3[:], rhs=Sp[:],
                                             start=(t == 0), stop=(t == NT - 1))

            # ================= FINAL: og norm + FFN + softmax =================
            with tc.tile_pool(name="fin", bufs=1) as fin, \
                 tc.tile_pool(name="finp", bufs=1, space="PSUM") as fnp:
                sq = fin.tile([GPC, EDGE_DIM - 1], F32, tag="sq")
                nc.scalar.activation(sq[:], og_raw[:], SQUARE)
                ss = fin.tile([GPC, 1], F32, tag="ss")
                nc.vector.tensor_reduce(out=ss[:], in_=sq[:], axis=mybir.AxisListType.X, op=ADD)
                nc.vector.tensor_scalar(ss[:], ss[:], 1e-24, None, op0=MAX)
                iss = fin.tile([GPC, 1], F32, tag="iss")
                nc.vector.reciprocal(iss[:], ss[:])
                rs = fin.tile([GPC, 1], F32, tag="rs")
                nc.scalar.activation(rs[:], iss[:], SQRT)
                ogn = fin.tile([GPC, EDGE_DIM - 1], F32, tag="ogn")
                nc.vector.tensor_scalar(ogn[:], og_raw[:], rs[:], None, op0=MULT)
                psum_ogT = fnp.tile([EDGE_DIM - 1, GPC], F32, tag="ogT")
                nc.tensor.transpose(psum_ogT[:], ogn[:], ident_f32[:GPC, :GPC])
                ogT = fin.tile([EDGE_DIM - 1, GPC], F32, tag="ogTs")
                nc.scalar.copy(ogT[:], psum_ogT[:])
                pooledT = fin.tile([H2, GPC], F32, tag="pooledT")
                nc.scalar.copy(pooledT[:], psum_pool[:])
                psum_z1 = fnp.tile([67, GPC], F32, tag="z1")
                nc.tensor.matmul(psum_z1[:], lhsT=Wf1a_sb[:], rhs=pooledT[:], start=True, stop=False)
                nc.tensor.matmul(psum_z1[:], lhsT=Wf1b_sb[:], rhs=ogT[:], start=False, stop=True)
                z1 = fin.tile([67, GPC], F32, tag="z1s")
                nc.scalar.activation(z1[:], psum_z1[:], RELU, bias=bf1c_sb[:])
                psum_z2 = fnp.tile([NCLS, GPC], F32, tag="z2")
                nc.tensor.matmul(psum_z2[:], lhsT=Wf2_sb[:], rhs=z1[:], start=True, stop=True)
                z2b = fin.tile([NCLS, GPC], F32, tag="z2b")
                nc.scalar.activation(z2b[:], psum_z2[:], IDENT, bias=bf2c_sb[:])
                psum_z2T = fnp.tile([GPC, NCLS], F32, tag="z2T")
                nc.tensor.transpose(psum_z2T[:], z2b[:], ident_f32[:NCLS, :NCLS])
                e2 = fin.tile([GPC, NCLS], F32, tag="e2")
                nc.scalar.activation(e2[:], psum_z2T[:], EXP)
                s2 = fin.tile([GPC, 1], F32, tag="s2")
                nc.vector.tensor_reduce(out=s2[:], in_=e2[:], axis=mybir.AxisListType.X, op=ADD)
                r2 = fin.tile([GPC, 1], F32, tag="r2")
                nc.vector.reciprocal(r2[:], s2[:])
                o2 = fin.tile([GPC, NCLS], F32, tag="o2")
                nc.vector.tensor_scalar(o2[:], e2[:], r2[:], None, op0=MULT)
                nc.sync.dma_start(out_dram[:], o2[:])

    nc.compile()
    return nc


import contextlib


@contextlib.contextmanager
def _nullpool():
    yield None


def kernel(**inputs) -> np.ndarray:
    dims, const, percore, node_start = host_prep(inputs)
    nc = build_program(dims, const)
    in_maps = []
    for k in range(NCORES):
        m = {name: np.ascontiguousarray(arr) for name, arr in const.items()}
        m.update(
            eaT_stream=percore["eaT_stream"][k],
            ea_og=percore["ea_og"][k],
            gl_og=percore["gl_og"][k],
            idx_w=percore["idx_w"][k],
            dstl=percore["dstl"][k],
            x_T=percore["x_T"][k],
            gl_node=percore["gl_node"][k],
            inv_cnt=percore["inv_cnt"][k],
        )
        in_maps.append(m)
    trace = bool(int(os.environ.get("BASS_KERNEL_TRACE", "0")))
    if trace:
        try:
            import sys as _sys, types as _types
            if "antenv.axon_hooks" not in _sys.modules:
                _m = _types.ModuleType("antenv.axon_hooks")
                _h = [None]

                def _get():
                    if _h[0] is None:
                        from trn_agent_boot.trn_boot import _ntff_profile_via_ctypes
                        _h[0] = _ntff_profile_via_ctypes("/opt/axon/libaxon_pjrt.so")
                    return _h[0]

                _m.get_axon_ntff_profile_hook = _get
                _m.set_axon_ntff_profile_hook = lambda h: _h.__setitem__(0, h)
                _sys.modules["antenv.axon_hooks"] = _m
        except Exception:
            trace = False
    res = run_bass_kernel_spmd(nc, in_maps, core_ids=list(range(NCORES)), trace=trace)
    if trace and res.exec_time_ns is not None:
        print(f"HW exec time: {res.exec_time_ns} ns")
    out = np.zeros((G, NCLS), np.float32)
    for k in range(NCORES):
        out[k * GPC:(k + 1) * GPC] = np.asarray(res.results[k]["out_gc"], np.float32)
    return out


# revision 16
# speedup vs baseline: 1.5114x; 1.2287x over previous
"""Trainium2 Bass kernel for nn_GAT_mlp_fed_1gram (3-layer GAT + 1-gram + FFN).

Self-contained: host-side numpy prep (sharding/sorting/index build + small-weight
folding) + an 8-core SPMD Bass/Tile program (graph-parallel slabs, dma_gather of
projected node features from an AllGathered table, one-hot-matmul segment
softmax/scatter, local pooling + FFN), assembled back to the full [128, 2] output.

Algorithm notes (validated against the reference in numpy):
  - (ee*a_e).sum(-1) folds to edge_attr @ (We . a_e)  -> [72, heads] per layer
  - (xs*a_s).sum(-1) folds into the projection: h @ [W | W.As | W.Ad]
  - segment softmax without max-subtraction (alpha is O(1)), normalization by
    post-division:  out = (sum_e e^a * xs_src) / (sum_e e^a + 1e-16)
  - loop_attr @ Wae == segment_mean(edge alphas): self-loop alphas come from a
    segment-mean of the per-edge folded alphas (matmul commutes with seg-sum)
  - self-loops (edge_attr fill 'mean') handled as one identity-chunk per tile
  - one-hot scatter matrices S / S^T are built once in the prepass and cached
    in DRAM; all three layers stream them back
"""
import os
import numpy as np
import ml_dtypes

import concourse.bacc as bacc
import concourse.mybir as mybir
import concourse.tile as tile
from concourse.bass_utils import run_bass_kernel_spmd
from concourse.library_config import mlp as _mlp_lib

BF16 = ml_dtypes.bfloat16
F32 = mybir.dt.float32
BF = mybir.dt.bfloat16
I16 = mybir.dt.int16

N, E, G = 50000, 400000, 128
D_NODE, EDGE_DIM, HEADS = 64, 72, 4
H0, H1, H2 = 128, 128, 64
NCLS = 2
NEG = 0.2
NCORES = 8
GPC = G // NCORES
P = 128
BUCKET = 32768
AOFF = {1: 0, 2: 4, 3: 8}
LHEADS = {1: HEADS, 2: HEADS, 3: 1}
LC = {1: HEADS * H0, 2: HEADS * H1, 3: H2}
LROW = {1: 640, 2: 640, 3: 128}   # bf16 slots per table row (stride, 256B mult)
LAS = {1: 8, 2: 8, 3: 2}          # leading bf16 slots holding fp32 asrc
EXP = mybir.ActivationFunctionType.Exp
RELU = mybir.ActivationFunctionType.Relu
COPY = mybir.ActivationFunctionType.Copy
SQUARE = mybir.ActivationFunctionType.Square
SQRT = mybir.ActivationFunctionType.Sqrt
IDENT = mybir.ActivationFunctionType.Identity
EQ = mybir.AluOpType.is_equal
MULT = mybir.AluOpType.mult
ADD = mybir.AluOpType.add
MAX = mybir.AluOpType.max


def _wrap16(idx):
    """dma_gather idx layout: idx i -> [i%16, i//16], replicated to 128 partitions."""
    n = len(idx)
    assert n % 16 == 0
    w = np.zeros((16, n // 16), np.int16)
    w[np.arange(n) % 16, np.arange(n) // 16] = idx
    return np.tile(w, (8, 1))


def host_prep(inputs):
    x = np.ascontiguousarray(np.asarray(inputs["x"], np.float32))
    ei = np.asarray(inputs["edge_index"])
    ea = np.ascontiguousarray(np.asarray(inputs["edge_attr"], np.float32))
    batch = np.asarray(inputs["batch"]).astype(np.int64)
    src, dst = ei[0].astype(np.int64), ei[1].astype(np.int64)

    node_start = np.searchsorted(batch, np.arange(0, G + 1, GPC))
    NT = int(np.ceil(np.diff(node_start).max() / P))
    NMAX = NT * P
    core_of_node = np.searchsorted(node_start[1:], np.arange(N), side="right")
    local_of_node = np.arange(N) - node_start[core_of_node]
    table_row = core_of_node * NMAX + local_of_node

    e_core = core_of_node[dst]
    per_core = []
    CA_need = CB_need = 0
    for k in range(NCORES):
        sel = np.nonzero(e_core == k)[0]
        d_loc = local_of_node[dst[sel]]
        order = np.argsort(d_loc, kind="stable")
        sel, d_loc = sel[order], d_loc[order]
        s_row = table_row[src[sel]]
        per_core.append((sel, d_loc, s_row))
        t_of = d_loc // P
        for t in range(NT):
            m = t_of == t
            ca = int((s_row[m] < BUCKET).sum())
            CA_need = max(CA_need, ca)
            CB_need = max(CB_need, int(m.sum()) - ca)
    CPT_A = max(1, int(np.ceil(CA_need / P)))
    CPT_B = max(1, int(np.ceil(CB_need / P)))
    CPT = CPT_A + CPT_B
    CA, CB = CPT_A * P, CPT_B * P

    idx_w = np.zeros((NCORES, NT, 128, (CA + CB) // 16), np.int16)
    # pads get dstl=127.5: the one-hot S never matches them, so they
    # contribute nothing to any scatter matmul (no mask needed anywhere)
    dstl = np.full((NCORES, 128, NT * CPT), 127.5, np.float32)
    # eaT layout: per tile [73, CPT*128]; row 72 = all-ones indicator
    eaT_stream = np.zeros((NCORES, NT, EDGE_DIM + 1, CPT * P), BF16)
    eaT_stream[:, :, EDGE_DIM, :] = 1.0
    og_core = (batch[src] // GPC).astype(np.int64)
    NOG = max(int((og_core == k).sum()) for k in range(NCORES))
    NOG = int(np.ceil(NOG / (4 * P))) * 4 * P
    ea_og = np.zeros((NCORES, NOG // P, P, EDGE_DIM), BF16)
    gl_og = np.full((NCORES, 128, NOG // P), 200.0, np.float32)

    ea_bf = ea.astype(BF16)
    for k in range(NCORES):
        sel, d_loc, s_row = per_core[k]
        t_of = d_loc // P
        for t in range(NT):
            m = np.nonzero(t_of == t)[0]
            sa = m[s_row[m] < BUCKET]
            sb_ = m[s_row[m] >= BUCKET]
            ia = np.zeros(CA, np.int16)
            ib = np.zeros(CB, np.int16)
            ia[:len(sa)] = s_row[sa].astype(np.int16)
            ib[:len(sb_)] = (s_row[sb_] - BUCKET).astype(np.int16)
            idx_w[k, t] = np.concatenate([_wrap16(ia), _wrap16(ib)], 1)
            for c_off, rows in ((0, sa), (CA, sb_)):
                nn_ = len(rows)
                j = np.arange(nn_)
                cols = (t * CPT * P + c_off + j)
                dstl[k, (cols % P), (cols // P)] = (d_loc[rows] - t * P).astype(np.float32)
                eaT_stream[k, t, :EDGE_DIM, c_off + j] = ea_bf[sel[rows]]
        m = np.nonzero(og_core == k)[0]
        j = np.arange(len(m))
        ea_og[k, j // P, j % P] = ea_bf[m]
        gl_og[k, (j % P), (j // P)] = (batch[src[m]] - k * GPC).astype(np.float32)

    def fold(W, a_s, a_d, heads):
        Wr = np.asarray(W, np.float32).reshape(W.shape[0], heads, -1)
        return np.concatenate([np.einsum("dhc,hc->dh", Wr, np.asarray(a_s, np.float32)),
                               np.einsum("dhc,hc->dh", Wr, np.asarray(a_d, np.float32))], 1)

    W_ext = {
        1: np.concatenate([np.asarray(inputs["W1"], np.float32),
                           fold(inputs["W1"], inputs["as1"], inputs["ad1"], HEADS)], 1),
        2: np.concatenate([np.asarray(inputs["W2"], np.float32),
                           fold(inputs["W2"], inputs["as2"], inputs["ad2"], HEADS)], 1),
        3: np.concatenate([np.asarray(inputs["W3"], np.float32),
                           fold(inputs["W3"], inputs["as3"], inputs["ad3"], 1)], 1),
    }
    Wae0 = np.concatenate([
        np.einsum("dhc,hc->dh", np.asarray(inputs["We1"], np.float32).reshape(EDGE_DIM, HEADS, H0), np.asarray(inputs["ae1"], np.float32)),
        np.einsum("dhc,hc->dh", np.asarray(inputs["We2"], np.float32).reshape(EDGE_DIM, HEADS, H1), np.asarray(inputs["ae2"], np.float32)),
        np.einsum("dhc,hc->dh", np.asarray(inputs["We3"], np.float32).reshape(EDGE_DIM, 1, H2), np.asarray(inputs["ae3"], np.float32)),
    ], 1)  # [72, 9]
    # [73, 10]: rows 0..71 = folded edge-alpha weights, col 9 picks the
    # indicator row -> per-edge constant 1 (deg accumulates via S)
    Wae = np.zeros((EDGE_DIM + 1, 10), np.float32)
    Wae[:EDGE_DIM, :9] = Wae0
    Wae[EDGE_DIM, 9] = 1.0

    x_T = np.zeros((NCORES, D_NODE, NMAX), np.float32)
    gl_node = np.full((NCORES, 128, NT), 200.0, np.float32)
    inv_cnt = np.zeros((NCORES, 128, GPC), np.float32)
    for k in range(NCORES):
        n0, n1 = node_start[k], node_start[k + 1]
        x_T[k, :, :n1 - n0] = x[n0:n1].T
        loc = np.arange(n1 - n0)
        gl_node[k, loc % P, loc // P] = (batch[n0:n1] - k * GPC).astype(np.float32)
        cnt = np.bincount(batch[n0:n1] - k * GPC, minlength=GPC).astype(np.float32)
        inv_cnt[k] = np.tile(1.0 / np.maximum(cnt, 1.0), (P, 1))

    const = dict(
        iota_row=np.tile(np.arange(P, dtype=np.float32), (P, 1)),
        iota16=np.tile(np.arange(GPC, dtype=np.float32), (P, 1)),
        ident_bf=np.eye(P, dtype=np.float32).astype(BF16),
        ident_f32=np.eye(P, dtype=np.float32),
        Wae=Wae.astype(BF16),
        W1ext=W_ext[1].astype(np.float32),
        W2ext=W_ext[2].reshape(4, 128, 520).transpose(1, 0, 2).reshape(128, 4 * 520).astype(BF16),
        W3ext=W_ext[3].reshape(4, 128, 66).transpose(1, 0, 2).reshape(128, 4 * 66).astype(BF16),
        B1=np.tile(np.asarray(inputs["b1"], np.float32), (P, 1)),
        B2=np.tile(np.asarray(inputs["b2"], np.float32), (P, 1)),
        B3=np.tile(np.asarray(inputs["b3"], np.float32), (P, 1)),
        Wf1a=np.asarray(inputs["Wf1"], np.float32)[:H2],
        Wf1b=np.asarray(inputs["Wf1"], np.float32)[H2:],
        Wf2=np.asarray(inputs["Wf2"], np.float32),
        bf1c=np.asarray(inputs["bf1"], np.float32)[:, None],
        bf2c=np.asarray(inputs["bf2"], np.float32)[:, None],
    )
    dims = dict(NT=NT, NMAX=NMAX, CPT_A=CPT_A, CPT_B=CPT_B, CPT=CPT, CA=CA, CB=CB, NOG=NOG)
    percore = dict(idx_w=idx_w, dstl=dstl, eaT_stream=eaT_stream,
                   ea_og=ea_og, gl_og=gl_og, x_T=x_T, gl_node=gl_node, inv_cnt=inv_cnt)
    return dims, const, percore, node_start


def build_program(dims, const):
    NT, NMAX = dims["NT"], dims["NMAX"]
    CPT_A, CPT_B, CPT = dims["CPT_A"], dims["CPT_B"], dims["CPT"]
    CA, CB = dims["CA"], dims["CB"]
    NOG = dims["NOG"]
    NOGC = NOG // P
    IDXW = (CA + CB) // 16
    SB = CPT * P                       # S-block width per tile

    nc = bacc.Bacc("TRN2", target_bir_lowering=False, debug=False, num_devices=NCORES)

    din = {}
    def dram_in(name, shape, dt=F32):
        din[name] = nc.dram_tensor(name, list(shape), dt, kind="ExternalInput")
        return din[name]

    eaT_dram = dram_in("eaT_stream", [NT, EDGE_DIM + 1, CPT * P], BF)
    ea_og_dram = dram_in("ea_og", [NOGC, P, EDGE_DIM], BF)
    gl_og_dram = dram_in("gl_og", [P, NOGC])
    idx_dram = dram_in("idx_w", [NT, P, IDXW], I16)
    dstl_dram = dram_in("dstl", [P, NT * CPT])
    xT_dram = dram_in("x_T", [D_NODE, NMAX])
    gl_node_dram = dram_in("gl_node", [P, NT])
    inv_cnt_dram = dram_in("inv_cnt", [P, GPC])
    for cname, arr in const.items():
        dram_in(cname, arr.shape, BF if arr.dtype == BF16 else F32)

    out_dram = nc.dram_tensor("out_gc", [GPC, NCLS], F32, kind="ExternalOutput")

    ag_in = {l: nc.dram_tensor(f"ag_in{l}", [NMAX, LROW[l]], BF, kind="Internal")
             for l in (1, 2, 3)}
    table = {l: nc.dram_tensor(f"table{l}", [NCORES * NMAX, LROW[l]], BF,
                               kind="Internal", addr_space="Shared") for l in (1, 2, 3)}
    s_dram = nc.dram_tensor("s_blocks", [NT, P, SB], BF, kind="Internal")
    st_dram = nc.dram_tensor("st_blocks", [NT, P, SB], BF, kind="Internal")

    RG = [list(range(NCORES))]

    with tile.TileContext(nc) as tc:
        nc.gpsimd.load_library(_mlp_lib)
        import contextlib
        ctx = contextlib.ExitStack()
        with ctx:
            persist = ctx.enter_context(tc.tile_pool(name="persist", bufs=1))

            def pload(name, shape=None, dt=F32):
                t = persist.tile(list(shape if shape is not None else const[name].shape), dt, tag=name)
                nc.sync.dma_start(t[:], din[name][:])
                return t

            iota_row = pload("iota_row")
            iota16 = pload("iota16")
            ident_bf = pload("ident_bf", dt=BF)
            ident_f32 = pload("ident_f32")
            Wae_sb = pload("Wae", dt=BF)
            W1ext_sb = pload("W1ext")
            W2ext_sb = pload("W2ext", dt=BF)
            W3ext_sb = pload("W3ext", dt=BF)
            B_sb = {1: pload("B1"), 2: pload("B2"), 3: pload("B3")}
            Wf1a_sb = pload("Wf1a"); Wf1b_sb = pload("Wf1b"); Wf2_sb = pload("Wf2")
            bf1c_sb = pload("bf1c"); bf2c_sb = pload("bf2c")
            dstl_sb = persist.tile([P, NT * CPT], F32, tag="dstl")
            nc.sync.dma_start(dstl_sb[:], dstl_dram[:])
            idx_sb = persist.tile([P, NT * IDXW], I16, tag="idx")
            nc.sync.dma_start(idx_sb[:].rearrange("p (t k) -> p t k", t=NT),
                              idx_dram[:].rearrange("t p k -> p t k"))
            gl_node_sb = persist.tile([P, NT], F32, tag="gl_node")
            nc.sync.dma_start(gl_node_sb[:], gl_node_dram[:])
            inv_cnt_sb = persist.tile([P, GPC], F32, tag="inv_cnt")
            nc.sync.dma_start(inv_cnt_sb[:], inv_cnt_dram[:])
            gl_og_sb = persist.tile([P, NOGC], F32, tag="gl_og")
            nc.sync.dma_start(gl_og_sb[:], gl_og_dram[:])

            alpha_e_sb = persist.tile([P, NT * CPT * 9], F32, tag="alpha_e")
            alpha_loop_sb = persist.tile([P, NT * 9], F32, tag="alpha_loop")
            asd_own = persist.tile([P, NT * 8], F32, tag="asd_own")
            asum_own = persist.tile([P, NT * 4], F32, tag="asum_own")
            ad_bf = persist.tile([P, NT * 4], BF, tag="ad_bf")
            h_slab = persist.tile([P, NT * 512], BF, tag="h_slab")
            og_raw = persist.tile([GPC, EDGE_DIM - 1], F32, tag="og_raw")

            def emit_og():
                with tc.tile_pool(name="ogp", bufs=3) as pre, \
                     tc.tile_pool(name="ogpsum", bufs=1, space="PSUM") as ogp:
                    psum_og = ogp.tile([GPC, EDGE_DIM - 1], F32, tag="og")
                    OGB = 4
                    for ob in range(NOGC // OGB):
                        eo = pre.tile([P, OGB, EDGE_DIM], BF, tag="eo")
                        nc.sync.dma_start(eo[:], ea_og_dram[:].rearrange("a p d -> p a d")[:, ob * OGB:(ob + 1) * OGB, :])
                        for a in range(OGB):
                            oc = ob * OGB + a
                            Sog = pre.tile([P, GPC], BF, tag="sog")
                            nc.vector.tensor_scalar(Sog[:], iota16[:], gl_og_sb[:, oc:oc + 1], None, op0=EQ)
                            nc.tensor.matmul(psum_og[:], lhsT=Sog[:], rhs=eo[:, a, :EDGE_DIM - 1],
                                             start=(oc == 0), stop=(oc == NOGC - 1))
                    nc.scalar.copy(og_raw[:], psum_og[:])

            def emit_prepass():
                with tc.tile_pool(name="pre", bufs=3) as pre, \
                     tc.tile_pool(name="sblk", bufs=2) as sblk, \
                     tc.tile_pool(name="prepsum", bufs=2, space="PSUM") as pps:
                    for t in range(NT):
                        # --- build S block + S^T block, cache to DRAM ---
                        s_blk = sblk.tile([P, SB], BF, tag="s")
                        st_blk = sblk.tile([P, SB], BF, tag="st")
                        for c in range(CPT):
                            tcn = t * CPT + c
                            nc.vector.tensor_scalar(s_blk[:, c * P:(c + 1) * P], iota_row[:],
                                                    dstl_sb[:, tcn:tcn + 1], None, op0=EQ)
                            psum_ST = pps.tile([P, P], BF, tag="stp")
                            nc.tensor.transpose(psum_ST[:], s_blk[:, c * P:(c + 1) * P], ident_bf[:])
                            nc.scalar.copy(st_blk[:, c * P:(c + 1) * P], psum_ST[:])
                        nc.sync.dma_start(s_dram[t], s_blk[:])
                        nc.sync.dma_start(st_dram[t], st_blk[:])
                        # --- per-chunk folded edge alphas + segment-mean for loops ---
                        psum_agg = pps.tile([P, 10], F32, tag="agg")
                        eaT = pre.tile([EDGE_DIM + 1, CPT * P], BF, tag="eaT")
                        nc.sync.dma_start(eaT[:], eaT_dram[t])
                        for c in range(CPT):
                            tcn = t * CPT + c
                            psum_ae = pps.tile([P, 10], F32, tag="ae")
                            nc.tensor.matmul(psum_ae[:], lhsT=eaT[:, c * P:(c + 1) * P],
                                             rhs=Wae_sb[:], start=True, stop=True)
                            nc.scalar.copy(alpha_e_sb[:, tcn * 9:(tcn + 1) * 9], psum_ae[:, :9])
                            aggrhs = pre.tile([P, 10], BF, tag="aggrhs")
                            nc.vector.tensor_copy(aggrhs[:], psum_ae[:])
                            nc.tensor.matmul(psum_agg[:], lhsT=s_blk[:, c * P:(c + 1) * P],
                                             rhs=aggrhs[:], start=(c == 0), stop=(c == CPT - 1))
                        dmax = pre.tile([P, 1], F32, tag="dmax")
                        nc.vector.tensor_scalar(dmax[:], psum_agg[:, 9:10], 1.0, None, op0=MAX)
                        rd = pre.tile([P, 1], F32, tag="rd")
                        nc.vector.reciprocal(rd[:], dmax[:])
                        nc.vector.tensor_scalar(alpha_loop_sb[:, t * 9:(t + 1) * 9],
                                                psum_agg[:, :9], rd[:], None, op0=MULT)

            # ================= LAYERS =================
            ppool = ctx.enter_context(tc.tile_pool(name="poolp", bufs=1, space="PSUM"))
            psum_pool = ppool.tile([H2, GPC], F32, tag="pool")

            def emit_proj(l):
                heads, C, ROW, ASL = LHEADS[l], LC[l], LROW[l], LAS[l]
                # ---------- projection -> ag_in[l] ----------
                with tc.tile_pool(name=f"proj{l}", bufs=2) as pj, \
                     tc.tile_pool(name=f"projp{l}", bufs=2, space="PSUM") as pjp:
                    for t in range(NT):
                        psum_x = pjp.tile([P, C], F32, tag="px")
                        psum_a = pjp.tile([P, 2 * heads], F32, tag="pa")
                        if l == 1:
                            xt = pj.tile([D_NODE, P], F32, tag="xt")
                            nc.sync.dma_start(xt[:], xT_dram[:, t * P:(t + 1) * P])
                            nc.tensor.matmul(psum_x[:], lhsT=xt[:], rhs=W1ext_sb[:, :C], start=True, stop=True)
                            nc.tensor.matmul(psum_a[:], lhsT=xt[:], rhs=W1ext_sb[:, C:C + 2 * heads], start=True, stop=True)
                        else:
                            Wsb = W2ext_sb if l == 2 else W3ext_sb
                            WR = 520 if l == 2 else 66
                            for kb in range(4):
                                psum_hT = pjp.tile([P, P], BF, tag="phT")
                                nc.tensor.transpose(psum_hT[:], h_slab[:, t * 512 + kb * 128: t * 512 + (kb + 1) * 128], ident_bf[:])
                                hT = pj.tile([P, P], BF, tag="hT")
                                nc.scalar.copy(hT[:], psum_hT[:])
                                nc.tensor.matmul(psum_x[:], lhsT=hT[:], rhs=Wsb[:, kb * WR:kb * WR + C],
                                                 start=(kb == 0), stop=(kb == 3))
                                nc.tensor.matmul(psum_a[:], lhsT=hT[:], rhs=Wsb[:, kb * WR + C:kb * WR + C + 2 * heads],
                                                 start=(kb == 0), stop=(kb == 3))
                        nc.scalar.copy(asd_own[:, t * 8:t * 8 + heads], psum_a[:, :heads])
                        nc.scalar.copy(asd_own[:, t * 8 + 4:t * 8 + 4 + heads], psum_a[:, heads:2 * heads])
                        nc.vector.tensor_copy(ad_bf[:, t * 4:t * 4 + heads], psum_a[:, heads:2 * heads])
                        row = pj.tile([P, ROW], BF, tag="row")
                        row_f32 = row[:].bitcast(F32)
                        nc.vector.tensor_copy(row_f32[:, :heads], psum_a[:, :heads])
                        nc.scalar.copy(row[:, ASL:ASL + C], psum_x[:])
                        nc.sync.dma_start(ag_in[l][t * P:(t + 1) * P, :], row[:])
                    nc.vector.tensor_tensor(
                        out=asum_own[:].rearrange("p (t k) -> p t k", k=4)[:, :, :heads],
                        in0=asd_own[:].rearrange("p (t k) -> p t k", k=8)[:, :, :heads],
                        in1=asd_own[:].rearrange("p (t k) -> p t k", k=8)[:, :, 4:4 + heads],
                        op=ADD)

            def emit_ag(l):
                nc.gpsimd.collective_compute(
                    "AllGather", mybir.AluOpType.bypass, replica_groups=RG,
                    ins=[ag_in[l][:]], outs=[table[l][:]],
                )

            def emit_main(l):
                heads, C, ROW, ASL = LHEADS[l], LC[l], LROW[l], LAS[l]
                HW = C // heads
                NCHUNK = CPT + 1
                AW = heads * NCHUNK
                with tc.tile_pool(name=f"main{l}", bufs=(4 if l == 3 else 3)) as mn, \
                     tc.tile_pool(name=f"mainp{l}", bufs=2, space="PSUM") as mp:
                    for t in range(NT):
                        gbuf = mn.tile([P, NCHUNK * ROW], BF, tag="gbuf")
                        nc.gpsimd.dma_gather(
                            gbuf[:, :CPT_A * ROW].rearrange("p (c e) -> p c e", e=ROW),
                            table[l][:], idx_sb[:, t * IDXW: t * IDXW + CA // 16],
                            CA, CA, ROW)
                        nc.gpsimd.dma_gather(
                            gbuf[:, CPT_A * ROW:CPT * ROW].rearrange("p (c e) -> p c e", e=ROW),
                            table[l][BUCKET:, :], idx_sb[:, t * IDXW + CA // 16: t * IDXW + IDXW],
                            CB, CB, ROW)
                        nc.sync.dma_start(gbuf[:, CPT * ROW:], ag_in[l][t * P:(t + 1) * P, :])
                        s_blk = mn.tile([P, SB], BF, tag="sblk")
                        nc.sync.dma_start(s_blk[:], s_dram[t])
                        st_blk = mn.tile([P, SB], BF, tag="stblk")
                        nc.sync.dma_start(st_blk[:], st_dram[t])

                        psum_za = mp.tile([P, AW + heads], F32, tag="za")
                        for c in range(CPT):
                            nc.tensor.matmul(psum_za[:, heads + c * heads: heads + (c + 1) * heads],
                                             lhsT=st_blk[:, c * P:(c + 1) * P],
                                             rhs=ad_bf[:, t * 4:t * 4 + heads],
                                             start=True, stop=True)
                        # ---- alpha assembly ----
                        t_al = mn.tile([P, AW], F32, tag="t_al")
                        gb_f32 = gbuf[:].bitcast(F32).rearrange("p (c e) -> p c e", e=ROW // 2)
                        nc.vector.tensor_tensor(
                            out=t_al[:].rearrange("p (c k) -> p c k", k=heads)[:, :CPT, :],
                            in0=gb_f32[:, :CPT, :heads],
                            in1=alpha_e_sb[:, t * CPT * 9:(t + 1) * CPT * 9].rearrange(
                                "p (c k) -> p c k", k=9)[:, :, AOFF[l]:AOFF[l] + heads],
                            op=ADD)
                        nc.vector.tensor_tensor(
                            out=t_al[:, CPT * heads:],
                            in0=asum_own[:, t * 4:t * 4 + heads],
                            in1=alpha_loop_sb[:, t * 9 + AOFF[l]: t * 9 + AOFF[l] + heads],
                            op=ADD)
                        nc.vector.tensor_tensor(out=t_al[:, :CPT * heads], in0=t_al[:, :CPT * heads],
                                                in1=psum_za[:, heads:heads + CPT * heads], op=ADD)
                        t_lr = mn.tile([P, AW], F32, tag="t_lr")
                        nc.vector.tensor_scalar(t_lr[:], t_al[:], NEG, None, op0=MULT)
                        nc.vector.tensor_tensor(out=t_al[:], in0=t_al[:], in1=t_lr[:], op=MAX)
                        p_bf = mn.tile([P, AW], BF, tag="p_bf")
                        nc.scalar.activation(p_bf[:], t_al[:], EXP)
                        # ---- messages + scatter (Z fused into M as extra cols) ----
                        if l < 3:
                            psum_M1 = mp.tile([P, 256], F32, tag="M1")
                            psum_M2 = mp.tile([P, 260], F32, tag="M2")
                        else:
                            psum_M1 = mp.tile([P, C + 1], F32, tag="M1")
                        for c in range(NCHUNK):
                            Sw = s_blk[:, c * P:(c + 1) * P] if c < CPT else ident_bf[:]
                            g_xs = gbuf[:, c * ROW + ASL: c * ROW + ASL + C]
                            if l < 3:
                                m_t = mn.tile([P, 516], BF, tag="m")
                                nc.vector.tensor_tensor(
                                    out=m_t[:, :512].rearrange("p (a b) -> p a b", b=HW),
                                    in0=g_xs[:].rearrange("p (a b) -> p a b", b=HW),
                                    in1=p_bf[:, c * heads:(c + 1) * heads].rearrange("p (a b) -> p a b", b=1).to_broadcast([P, heads, HW]),
                                    op=MULT)
                                nc.scalar.copy(m_t[:, 512:516], p_bf[:, c * heads:(c + 1) * heads])
                                nc.tensor.matmul(psum_M1[:], lhsT=Sw, rhs=m_t[:, :256],
                                                 start=(c == 0), stop=(c == NCHUNK - 1))
                                nc.tensor.matmul(psum_M2[:], lhsT=Sw, rhs=m_t[:, 256:516],
                                                 start=(c == 0), stop=(c == NCHUNK - 1))
                            else:
                                m_t = mn.tile([P, C + 1], BF, tag="m")
                                nc.vector.tensor_tensor(
                                    out=m_t[:, :C],
                                    in0=g_xs[:],
                                    in1=p_bf[:, c:c + 1].to_broadcast([P, C]),
                                    op=MULT)
                                nc.scalar.copy(m_t[:, C:C + 1], p_bf[:, c:c + 1])
                                nc.tensor.matmul(psum_M1[:], lhsT=Sw, rhs=m_t[:],
                                                 start=(c == 0), stop=(c == NCHUNK - 1))
                        # ---- epilogue ----
                        if l < 3:
                            zt = mn.tile([P, heads], F32, tag="zt")
                            nc.vector.tensor_scalar(zt[:], psum_M2[:, 256:260], 1e-16, None, op0=ADD)
                            rz = mn.tile([P, heads], F32, tag="rz")
                            nc.vector.reciprocal(rz[:], zt[:])
                            ht = mn.tile([P, C], F32, tag="ht")
                            nc.vector.tensor_tensor(
                                out=ht[:, :256].rearrange("p (a b) -> p a b", b=HW),
                                in0=psum_M1[:, :256].rearrange("p (a b) -> p a b", b=HW),
                                in1=rz[:, 0:2].rearrange("p (a b) -> p a b", b=1).to_broadcast([P, 2, HW]), op=MULT)
                            nc.vector.tensor_tensor(
                                out=ht[:, 256:512].rearrange("p (a b) -> p a b", b=HW),
                                in0=psum_M2[:, :256].rearrange("p (a b) -> p a b", b=HW),
                                in1=rz[:, 2:4].rearrange("p (a b) -> p a b", b=1).to_broadcast([P, 2, HW]), op=MULT)
                            nc.vector.tensor_tensor(out=ht[:], in0=ht[:], in1=B_sb[l][:, :C], op=ADD)
                            nc.scalar.activation(h_slab[:, t * 512:(t + 1) * 512], ht[:], RELU)
                        else:
                            zt = mn.tile([P, 1], F32, tag="zt")
                            nc.vector.tensor_scalar(zt[:], psum_M1[:, C:C + 1], 1e-16, None, op0=ADD)
                            rz = mn.tile([P, 1], F32, tag="rz")
                            nc.vector.reciprocal(rz[:], zt[:])
                            ht = mn.tile([P, C], F32, tag="ht")
                            nc.vector.tensor_scalar(ht[:], psum_M1[:, :C], rz[:], None, op0=MULT)
                            nc.vector.tensor_tensor(out=ht[:], in0=ht[:], in1=B_sb[3][:, :C], op=ADD)
                            h3 = mn.tile([P, C], F32, tag="h3")
                            nc.scalar.activation(h3[:], ht[:], RELU)
                            Sp = mn.tile([P, GPC], F32, tag="Sp")
                            nc.vector.tensor_scalar(Sp[:], iota16[:], gl_node_sb[:, t:t + 1], None, op0=EQ)
                            nc.vector.tensor_tensor(out=Sp[:], in0=Sp[:], in1=inv_cnt_sb[:], op=MULT)
                            nc.tensor.matmul(psum_pool[:], lhsT=h3[:], rhs=Sp[:],
                                             start=(t == 0), stop=(t == NT - 1))

            emit_proj(1)
            emit_ag(1)
            emit_prepass()
            emit_main(1)
            emit_proj(2)
            emit_ag(2)
            emit_og()
            emit_main(2)
            emit_proj(3)
            emit_ag(3)
            emit_main(3)

            # ================= FINAL: og norm + FFN + softmax =================
            with tc.tile_pool(name="fin", bufs=1) as fin, \
                 tc.tile_pool(name="finp", bufs=1, space="PSUM") as fnp:
                sq = fin.tile([GPC, EDGE_DIM - 1], F32, tag="sq")
                nc.scalar.activation(sq[:], og_raw[:], SQUARE)
                ss = fin.tile([GPC, 1], F32, tag="ss")
                nc.vector.tensor_reduce(out=ss[:], in_=sq[:], axis=mybir.AxisListType.X, op=ADD)
                nc.vector.tensor_scalar(ss[:], ss[:], 1e-24, None, op0=MAX)
                iss = fin.tile([GPC, 1], F32, tag="iss")
                nc.vector.reciprocal(iss[:], ss[:])
                rs = fin.tile([GPC, 1], F32, tag="rs")
                nc.scalar.activation(rs[:], iss[:], SQRT)
                ogn = fin.tile([GPC, EDGE_DIM - 1], F32, tag="ogn")
                nc.vector.tensor_scalar(ogn[:], og_raw[:], rs[:], None, op0=MULT)
                psum_ogT = fnp.tile([EDGE_DIM - 1, GPC], F32, tag="ogT")
                nc.tensor.transpose(psum_ogT[:], ogn[:], ident_f32[:GPC, :GPC])
                ogT = fin.tile([EDGE_DIM - 1, GPC], F32, tag="ogTs")
                nc.scalar.copy(ogT[:], psum_ogT[:])
                pooledT = fin.tile([H2, GPC], F32, tag="pooledT")
                nc.scalar.copy(pooledT[:], psum_pool[:])
                psum_z1 = fnp.tile([67, GPC], F32, tag="z1")
                nc.tensor.matmul(psum_z1[:], lhsT=Wf1a_sb[:], rhs=pooledT[:], start=True, stop=False)
                nc.tensor.matmul(psum_z1[:], lhsT=Wf1b_sb[:], rhs=ogT[:], start=False, stop=True)
                z1 = fin.tile([67, GPC], F32, tag="z1s")
                nc.scalar.activation(z1[:], psum_z1[:], RELU, bias=bf1c_sb[:])
                psum_z2 = fnp.tile([NCLS, GPC], F32, tag="z2")
                nc.tensor.matmul(psum_z2[:], lhsT=Wf2_sb[:], rhs=z1[:], start=True, stop=True)
                z2b = fin.tile([NCLS, GPC], F32, tag="z2b")
                nc.scalar.activation(z2b[:], psum_z2[:], IDENT, bias=bf2c_sb[:])
                psum_z2T = fnp.tile([GPC, NCLS], F32, tag="z2T")
                nc.tensor.transpose(psum_z2T[:], z2b[:], ident_f32[:NCLS, :NCLS])
                e2 = fin.tile([GPC, NCLS], F32, tag="e2")
                nc.scalar.activation(e2[:], psum_z2T[:], EXP)
                s2 = fin.tile([GPC, 1], F32, tag="s2")
                nc.vector.tensor_reduce(out=s2[:], in_=e2[:], axis=mybir.AxisListType.X, op=ADD)
                r2 = fin.tile([GPC, 1], F32, tag="r2")
                nc.vector.reciprocal(r2[:], s2[:])
                o2 = fin.tile([GPC, NCLS], F32, tag="o2")
                nc.vector.tensor_scalar(o2[:], e2[:], r2[:], None, op0=MULT)
                nc.sync.dma_start(out_dram[:], o2[:])

    nc.compile()
    return nc


import contextlib


@contextlib.contextmanager
def _nullpool():
    yield None


def kernel(**inputs) -> np.ndarray:
    dims, const, percore, node_start = host_prep(inputs)
    nc = build_program(dims, const)
    in_maps = []
    for k in range(NCORES):
        m = {name: np.ascontiguousarray(arr) for name, arr in const.items()}
        m.update(
            eaT_stream=percore["eaT_stream"][k],
            ea_og=percore["ea_og"][k],
            gl_og=percore["gl_og"][k],
            idx_w=percore["idx_w"][k],
            dstl=percore["dstl"][k],
            x_T=percore["x_T"][k],
            gl_node=percore["gl_node"][k],
            inv_cnt=percore["inv_cnt"][k],
        )
        in_maps.append(m)
    trace = bool(int(os.environ.get("BASS_KERNEL_TRACE", "0")))
    if trace:
        try:
            import sys as _sys, types as _types
            if "antenv.axon_hooks" not in _sys.modules:
                _m = _types.ModuleType("antenv.axon_hooks")
                _h = [None]

                def _get():
                    if _h[0] is None:
                        from trn_agent_boot.trn_boot import _ntff_profile_via_ctypes
                        _h[0] = _ntff_profile_via_ctypes("/opt/axon/libaxon_pjrt.so")
                    return _h[0]

                _m.get_axon_ntff_profile_hook = _get
                _m.set_axon_ntff_profile_hook = lambda h: _h.__setitem__(0, h)
                _sys.modules["antenv.axon_hooks"] = _m
        except Exception:
            trace = False
    res = run_bass_kernel_spmd(nc, in_maps, core_ids=list(range(NCORES)), trace=trace)
    if trace and res.exec_time_ns is not None:
        print(f"HW exec time: {res.exec_time_ns} ns")
    out = np.zeros((G, NCLS), np.float32)
    for k in range(NCORES):
        out[k * GPC:(k + 1) * GPC] = np.asarray(res.results[k]["out_gc"], np.float32)
    return out
